# revision 9
# baseline (speedup 1.0000x reference)
"""CrossKD loss kernel for Trainium2, 8 NeuronCores — v4.

Sharding: one (image, scale) pair per core. Cores 0-3: scale-0 images
(2048 anchors); cores 4-7: scale-1 images (1024 anchors) padded to 2048
students with inert rows. One SPMD program on all 8 cores.

v4 changes vs v3 (239us):
  * Used-mask folded into the Ln denominator: a PSUM tensor
    Uta = ta + 65504*(#kills) is maintained by the PE kill-matmuls, and
    lnS = Ln(Uta + sa) de-ranks killed columns (ln(65504) ~ 11 pushes
    them far below the -1.1 match threshold, and they can never pass it
    since lni <= ~0.8). This deletes both the per-stage U16 scalar copy
    and the full-width av add on vector.
  * W dynamic: ceil64(max valid teachers) instead of hardcoded 1152.
  * Teacher coord rows (tx1,tx2,ty1,ty2,iota) shipped pre-replicated
    [128,5,W] f16 over DMA — the startup PE/scalar replicate cascade is
    gone; only ta + invalid-mask enter via K=1 matmuls (Uta init).
  * x-axis intersection via Scalar relu-form: whx = relu(sw - a - b),
    a = relu(tx1-sx1), b = relu(sx2-tx2) — two TS + one TT leave the
    (bottleneck) vector engine for the (slack) scalar engine.
  * KL loss computed in 4-tile chunks pumped as fill work inside the
    stage loop (chunks 0-2); only the last chunk runs in the epilogue.
Host: sums the 4 accumulators over 8 cores, normalizes, weighted sum.
"""
import numpy as np

ALPHA, BETA, TEMP = 0.6, 0.3, 4.0
LN_THR = -1.0986122886681098   # iou > 0.5  <=>  r > 1/3  <=>  ln r > ln(1/3)
KILLV = 65504.0                # f16 max; ln(ta+KILLV+sa) ~ 11.1 >> |LN_THR|
NS = 2048                # padded students per core
NT = 16                  # student tiles
D = 85

PAD_X = 30000.0          # inert-student x center (fp16-safe)

_CACHE = {}


def _build_nc(W):
    import concourse.bacc as bacc
    import concourse.mybir as mybir
    from concourse.tile import TileContext
    from concourse.alu_op_type import AluOpType as Op
    dt = mybir.dt
    AF = mybir.ActivationFunctionType
    AX = mybir.AxisListType
    f32 = dt.float32
    f16 = dt.float16

    nc = bacc.Bacc("TRN2", num_devices=8, debug=False)

    # ---- DRAM I/O ----
    tcoords = nc.dram_tensor("tcoords", [128, 5, W], f16, kind="ExternalInput")  # tx1,tx2,ty1,ty2,iota
    ta_row_d = nc.dram_tensor("ta_row", [1, W], f32, kind="ExternalInput")
    inv_row_d = nc.dram_tensor("inv_row", [1, W], f16, kind="ExternalInput")
    s_cols = nc.dram_tensor("s_cols", [128, NT, 5], f32, kind="ExternalInput")
    s_logits = nc.dram_tensor("s_logits", [128, NT, 80], f32, kind="ExternalInput")
    t_rows_nat = nc.dram_tensor("t_rows_nat", [W, D], f32, kind="ExternalInput")
    # packed f32 consts: ltmask | identity | p1col | ones128_col
    cpack = nc.dram_tensor("cpack", [128, 258], f32, kind="ExternalInput")
    ones_col = nc.dram_tensor("ones_col", [1, 128], f32, kind="ExternalInput")
    kbig_lhs = nc.dram_tensor("kbig_lhs", [128, 128], f16, kind="ExternalInput")  # 65504
    out = nc.dram_tensor("out", [1, 8], f32, kind="ExternalOutput")

    # PSUM-bank-aligned accumulation chunks
    CH = []
    o = 0
    while o < W:
        n = min(512, W - o)
        CH.append((o, n))
        o += n

    from contextlib import ExitStack
    with TileContext(nc) as tc, ExitStack() as stack:
        # Load the one activation table that serves every func used here
        # (exp, ln, relu, copy, abs) so the table-load pass never swaps.
        from concourse.hw_specs import get_activation_tables
        _tabs = list(get_activation_tables(nc.m.arch))
        nc.scalar.add_instruction(mybir.InstLoadActFuncSet(
            name=nc.scalar.bass.get_next_instruction_name(),
            act_func_set_id=_tabs.index("natural_log_exp_and_others")))
        sb = stack.enter_context(tc.tile_pool(name="sbp", bufs=1))
        ps = stack.enter_context(tc.tile_pool(name="ps", bufs=1, space="PSUM"))
        sbb = stack.enter_context(tc.tile_pool(name="sbb", bufs=2))
        sbr = stack.enter_context(tc.tile_pool(name="sbr", bufs=3))
        sbit = stack.enter_context(tc.tile_pool(name="sbit", bufs=2))

        # ---------- inputs (critical-path first, spread across DMA queues) ----------
        tco = sb.tile([128, 5, W], f16)
        nc.scalar.dma_start(tco[:, :, :], tcoords.ap()[:, :, :])
        s_c = sb.tile([128, NT, 5], f32)
        nc.scalar.dma_start(s_c[:, :, :], s_cols.ap()[:, :, :])
        c_ones1 = sb.tile([1, 128], f32); nc.sync.dma_start(c_ones1[:, :], ones_col.ap()[:, :])
        ta_row = sb.tile([1, W], f32)
        nc.sync.dma_start(ta_row[:1, :], ta_row_d.ap()[:, :])
        inv_row = sb.tile([1, W], f16)
        nc.sync.dma_start(inv_row[:1, :], inv_row_d.ap()[:, :])
        c_kbig = sb.tile([128, 128], f16); nc.sync.dma_start(c_kbig[:, :], kbig_lhs.ap()[:, :])
        c_all = sb.tile([128, 258], f32)
        nc.sync.dma_start(c_all[:, :], cpack.ap()[:, :])
        c_lt = c_all[:, 0:128]
        c_id = c_all[:, 128:256]
        c_p1 = c_all[:, 256:257]
        c_ones_col = c_all[:, 257:258]
        slg = sb.tile([128, NT, 80], f32)
        nc.gpsimd.dma_start(slg[:, :, :], s_logits.ap()[:, :, :])

        # ---------- Uta init: PSUM = ta + KILLV*invalid ----------
        Uta = ps.tile([128, W], f32, tag="ps_U", name="Uta")
        for (o, n) in CH:
            nc.tensor.matmul(Uta[:, o:o+n], c_ones1[:1, :], ta_row[0:1, o:o+n],
                             start=True, stop=True, skip_group_check=True)
        for (o, n) in CH:
            nc.tensor.matmul(Uta[:, o:o+n], c_kbig[0:1, :], inv_row[:1, o:o+n],
                             start=False, stop=True, skip_group_check=True)

        # ---------- student scalars [128, NT] ----------
        sx1 = sb.tile([128, NT], f32); sx2 = sb.tile([128, NT], f32)
        sy1 = sb.tile([128, NT], f32); sy2 = sb.tile([128, NT], f32)
        sa = sb.tile([128, NT], f32)
        nc.vector.scalar_tensor_tensor(sx1[:, :], s_c[:, :, 2], -0.5, s_c[:, :, 0], Op.mult, Op.add)
        nc.vector.scalar_tensor_tensor(sx2[:, :], s_c[:, :, 2], 0.5, s_c[:, :, 0], Op.mult, Op.add)
        nc.vector.scalar_tensor_tensor(sy1[:, :], s_c[:, :, 3], -0.5, s_c[:, :, 1], Op.mult, Op.add)
        nc.vector.scalar_tensor_tensor(sy2[:, :], s_c[:, :, 3], 0.5, s_c[:, :, 1], Op.mult, Op.add)
        nsx1 = sb.tile([128, NT], f32)
        nc.vector.tensor_scalar(nsx1[:, :], sx1[:, :], -1.0, None, Op.mult)
        tmpw = sb.tile([128, NT], f32)
        nc.vector.tensor_tensor(sa[:, :], sx2[:, :], sx1[:, :], Op.subtract)
        nc.vector.tensor_tensor(tmpw[:, :], sy2[:, :], sy1[:, :], Op.subtract)
        nc.vector.tensor_tensor(sa[:, :], sa[:, :], tmpw[:, :], Op.mult)
        sa1e7 = sb.tile([128, NT], f32)
        nc.vector.tensor_scalar(sa1e7[:, :], sa[:, :], 1e-7, None, Op.add)

        # ---------- per-stage results ----------
        w_all = sb.tile([128, NT], f32)
        Gs = [sb.tile([128, 4, D], f32, tag=f"G{c}", name=f"G{c}") for c in range(4)]
        se_all = sb.tile([128, NT], f32)
        tse_all = sb.tile([128, NT], f32)
        klA = sb.tile([128, NT], f32)
        klB = sb.tile([128, NT], f32)

        # ---------- production of lni tiles (fill work) ----------
        prod = {}

        def make_prod(j):
            st = {}

            def p_a():
                st["a"] = sbb.tile([128, W], f16, tag="pa", name=f"a{j}")
                nc.scalar.activation(st["a"][:, :], tco[:, 0, :], AF.Relu,
                                     bias=nsx1[:, j:j+1])

            def p_b():
                st["b"] = sbb.tile([128, W], f16, tag="pb", name=f"b{j}")
                nc.scalar.activation(st["b"][:, :], tco[:, 1, :], AF.Relu,
                                     scale=-1.0, bias=sx2[:, j:j+1])

            def p_cx():
                st["cx"] = sbb.tile([128, W], f16, tag="pcx", name=f"cx{j}")
                nc.vector.tensor_tensor(st["cx"][:, :], st["a"][:, :], st["b"][:, :], Op.add)

            def p_whx():
                st["whx"] = sbb.tile([128, W], f16, tag="pwhx", name=f"whx{j}")
                nc.scalar.activation(st["whx"][:, :], st["cx"][:, :], AF.Relu,
                                     scale=-1.0, bias=s_c[:, j, 2:3])

            def p_m1y():
                st["m1y"] = sbb.tile([128, W], f16, tag="pm1y", name=f"m1y{j}")
                nc.vector.tensor_scalar(st["m1y"][:, :], tco[:, 2, :], sy1[:, j:j+1], None, Op.max)

            def p_t1y():
                st["t1y"] = sbb.tile([128, W], f16, tag="pt1y", name=f"t1y{j}")
                nc.vector.tensor_scalar(st["t1y"][:, :], tco[:, 3, :], sy2[:, j:j+1], None, Op.min)

            def p_wyr():
                st["wyr"] = sbb.tile([128, W], f16, tag="pwyr", name=f"wyr{j}")
                nc.vector.tensor_tensor(st["wyr"][:, :], st["t1y"][:, :], st["m1y"][:, :], Op.subtract)

            def p_why():
                st["why"] = sbb.tile([128, W], f16, tag="pwhy", name=f"why{j}")
                nc.scalar.activation(st["why"][:, :], st["wyr"][:, :], AF.Relu)

            def p_inter():
                st["inter"] = sbb.tile([128, W], f16, tag="pinter", name=f"inter{j}")
                nc.vector.tensor_tensor(st["inter"][:, :], st["whx"][:, :], st["why"][:, :], Op.mult)

            def p_lni():
                prod[j] = sbr.tile([128, W], f16, tag="lni", name=f"lni{j}")
                nc.scalar.activation(prod[j][:, :], st["inter"][:, :], AF.Ln)

            return [p_a, p_b, p_m1y, p_t1y, p_cx, p_wyr, p_whx, p_why, p_inter, p_lni]

        # ---------- KL chunk closures (fill work; chunk c = tiles 4c..4c+3) ----------
        def make_kl(c):
            T = slice(4 * c, 4 * c + 4)
            st = {}

            def k_sexp():
                st["sex"] = sbb.tile([128, 4, 80], f32, tag="ksex", name=f"sex{c}")
                nc.scalar.activation(st["sex"][:, :, :], slg[:, T, :], AF.Exp, scale=1.0 / TEMP)

            def k_texp():
                st["tex"] = sbb.tile([128, 4, 80], f32, tag="ktex", name=f"tex{c}")
                nc.scalar.activation(st["tex"][:, :, :], Gs[c][:, :, 5:], AF.Exp, scale=1.0 / TEMP)

            def k_se():
                nc.vector.tensor_reduce(se_all[:, T], st["sex"][:, :, :], AX.X, Op.add)

            def k_tse():
                nc.vector.tensor_reduce(tse_all[:, T], st["tex"][:, :, :], AX.X, Op.add)

            def k_pa():
                st["pA"] = sbb.tile([128, 4, 80], f32, tag="kpA", name=f"pA{c}")
                nc.vector.tensor_tensor(st["pA"][:, :, :], st["tex"][:, :, :], Gs[c][:, :, 5:], Op.mult)

            def k_ka():
                nc.vector.tensor_reduce(klA[:, T], st["pA"][:, :, :], AX.X, Op.add)

            def k_pb():
                st["pB"] = sbb.tile([128, 4, 80], f32, tag="kpB", name=f"pB{c}")
                nc.vector.tensor_tensor(st["pB"][:, :, :], st["tex"][:, :, :], slg[:, T, :], Op.mult)

            def k_kb():
                nc.vector.tensor_reduce(klB[:, T], st["pB"][:, :, :], AX.X, Op.add)

            return [k_sexp, k_texp, k_se, k_tse, k_pa, k_ka, k_pb, k_kb]

        # ---------- work queue ----------
        from collections import deque
        work = deque()

        def pump(n):
            for _ in range(n):
                if work:
                    work.popleft()()
                else:
                    return

        def pump_all():
            while work:
                work.popleft()()

        # prime: production 0 fully, production 1 queued
        for fn in make_prod(0):
            fn()
        work.extend(make_prod(1))
        pump(4)

        # lnS_0 / t_0
        lnS_cur = {}
        t_cur = {}
        lnS_cur[0] = sbb.tile([128, W], f16, tag="lnS", name="lnS0")
        nc.scalar.activation(lnS_cur[0][:, :], Uta[:, :], AF.Ln, bias=sa1e7[:, 0:1])
        t_cur[0] = sbr.tile([128, W], f16, tag="tt", name="t0")
        nc.vector.tensor_tensor(t_cur[0][:, :], prod[0][:, :], lnS_cur[0][:, :], Op.subtract)

        import concourse.bass as bass_mod
        KL_AT = {5: 0, 9: 1, 13: 2}   # stage -> chunk emitted as fill

        # ---------- stages ----------
        for j in range(NT):
            if j + 2 < NT:
                work.extend(make_prod(j + 2))
            if j in KL_AT:
                work.extend(make_kl(KL_AT[j]))

            t_j = t_cur.pop(j)
            top8v = sbit.tile([128, 8], f16, tag="st_top8v")
            nc.vector.max(top8v[:, :], t_j[:, :])
            pos8 = sbit.tile([128, 8], mybir.dt.uint32, tag="st_pos8")
            nc.vector.max_index(pos8[:, :], top8v[:, :], t_j[:, :])

            tid = sbit.tile([128, 1], f32, tag="st_tid")
            nc.vector.tensor_copy(tid[:, :], pos8[:, 0:1])
            act = sbit.tile([128, 1], f32, tag="st_act")
            nc.vector.tensor_scalar(act[:, :], top8v[:, 0:1], float(LN_THR), None, Op.is_gt)
            te1 = sbit.tile([128, 1], f32, tag="st_te1")
            nc.vector.scalar_tensor_tensor(te1[:, :], tid[:, :], c_p1[:, 0:1], act[:, :], Op.add, Op.mult)
            tid_eff = sbit.tile([128, 1], f32, tag="st_tideff")
            nc.vector.tensor_scalar(tid_eff[:, :], te1[:, :], c_p1[:, 0:1], None, Op.subtract)
            # one-hot over W of this stage's proposals: every act=1 proposal's
            # teacher is accepted by its earliest proposer, so proposals and
            # accepts kill the same columns — Uta commits before the conflict
            # round trip resolves.
            # PE: proposal broadcast (transpose) + Uta kill commit + conflict matrix
            tp = ps.tile([128, 128], f32, tag="ps_tp", name="ittp")
            nc.tensor.transpose(tp[0:1, 0:128], tid_eff[:, 0:1], c_id[:, :])
            if j + 1 < NT:
                ohw = sbit.tile([128, W], f16, tag="st_ohw")
                nc.vector.tensor_scalar(ohw[:, :], tco[:, 4, :], tid_eff[:, 0:1], None, Op.is_equal)
                for (o, n) in CH:
                    nc.tensor.matmul(Uta[:, o:o+n], c_kbig[:, :], ohw[:, o:o+n],
                                     start=False, stop=True, skip_group_check=True)

            if j + 1 < NT:
                lnS_cur[j + 1] = sbb.tile([128, W], f16, tag="lnS", name=f"lnS{j+1}")
                nc.scalar.activation(lnS_cur[j + 1][:, :], Uta[:, :], AF.Ln, bias=sa1e7[:, j+1:j+2])

            itrow = sbit.tile([1, 128], f32, tag="st_itrow")
            nc.scalar.copy(itrow[:1, :], tp[0:1, 0:128])
            trep = ps.tile([128, 128], f32, tag="ps_trep", name="ittrep")
            nc.tensor.matmul(trep[:, :], c_ones1[:1, :], itrow[:1, :])

            if j + 1 < NT:
                # drain fill until lni_{j+1} is emitted, then chain t_{j+1}
                while work and (j + 1) not in prod:
                    work.popleft()()
                t_cur[j + 1] = sbr.tile([128, W], f16, tag="tt", name=f"t{j+1}")
                nc.vector.tensor_tensor(t_cur[j + 1][:, :], prod.pop(j + 1)[:, :],
                                        lnS_cur.pop(j + 1)[:, :], Op.subtract)

            # conflict: an earlier partition proposes the same teacher -> lost
            cnt = sbit.tile([128, 1], f32, tag="st_cnt")
            escr = sbit.tile([128, 128], f32, tag="st_escr")
            nc.vector.scalar_tensor_tensor(escr[:, :], trep[:, :], tid_eff[:, 0:1], c_lt[:, :],
                                           Op.is_equal, Op.mult, accum_out=cnt[:, 0:1])
            notlost = sbit.tile([128, 1], f32, tag="st_nl")
            nc.vector.tensor_scalar(notlost[:, :], cnt[:, :], 0.5, None, Op.is_le)
            nc.vector.tensor_tensor(w_all[:, j:j+1], act[:, :], notlost[:, :], Op.mult)
            tsp1 = sbit.tile([128, 1], f32, tag="st_tsp1")
            nc.vector.scalar_tensor_tensor(tsp1[:, :], tid[:, :], 1.0, w_all[:, j:j+1], Op.add, Op.mult)
            tid_sel = sbit.tile([128, 1], f32, tag="st_tidsel")
            nc.vector.tensor_scalar(tid_sel[:, :], tsp1[:, :], 1.0, None, Op.subtract)

            # gather matched teacher rows from DRAM by index (idle DMA engines)
            tidc = sbit.tile([128, 1], f32, tag="st_tidc")
            nc.gpsimd.tensor_scalar(tidc[:, :], tid_sel[:, :], 0.0, None, Op.max)
            tidi = sbit.tile([128, 1], mybir.dt.int32, tag="st_tidi")
            nc.gpsimd.tensor_copy(tidi[:, :], tidc[:, :])
            nc.gpsimd.indirect_dma_start(
                out=Gs[j // 4][:, j % 4, :], out_offset=None,
                in_=t_rows_nat.ap()[:, :],
                in_offset=bass_mod.IndirectOffsetOnAxis(ap=tidi[:, 0:1], axis=0),
            )

            if j + 1 < NT:
                pump(3)  # keep KL-chunk backlog from piling onto the chain
            else:
                pump_all()

        pump_all()

        # ---------- epilogue ----------
        sbe = stack.enter_context(tc.tile_pool(name="sbe", bufs=1))

        # last KL chunk
        for fn in make_kl(3):
            fn()

        # klD = ln se - ln tse
        lnse = sbe.tile([128, NT], f32)
        nc.scalar.activation(lnse[:, :], se_all[:, :], AF.Ln)
        lntse = sbe.tile([128, NT], f32)
        nc.scalar.activation(lntse[:, :], tse_all[:, :], AF.Ln)
        klD = sbe.tile([128, NT], f32)
        nc.vector.tensor_tensor(klD[:, :], lnse[:, :], lntse[:, :], Op.subtract)

        # kl = 0.25*(klA - klB)/tse + klD, weighted by w
        kl = sbe.tile([128, NT], f32)
        nc.vector.tensor_tensor(kl[:, :], klA[:, :], klB[:, :], Op.subtract)
        rtse = sbe.tile([128, NT], f32)
        nc.vector.reciprocal(rtse[:, :], tse_all[:, :])
        nc.vector.tensor_scalar(rtse[:, :], rtse[:, :], 1.0 / TEMP, None, Op.mult)
        nc.vector.tensor_tensor(kl[:, :], kl[:, :], rtse[:, :], Op.mult)
        nc.vector.tensor_tensor(kl[:, :], kl[:, :], klD[:, :], Op.add)
        nc.vector.tensor_tensor(kl[:, :], kl[:, :], w_all[:, :], Op.mult)

        # --- exact miou recompute + box/conf (chunked over the 4 G tiles) ---
        gx1 = sbe.tile([128, NT], f32); gx2 = sbe.tile([128, NT], f32)
        gy1 = sbe.tile([128, NT], f32); gy2 = sbe.tile([128, NT], f32)
        gta = sbe.tile([128, NT], f32)
        e1 = sbe.tile([128, NT], f32); e2 = sbe.tile([128, NT], f32)
        for c in range(4):
            T = slice(4 * c, 4 * c + 4)
            nc.vector.scalar_tensor_tensor(gx1[:, T], Gs[c][:, :, 2], -0.5, Gs[c][:, :, 0], Op.mult, Op.add)
            nc.vector.scalar_tensor_tensor(gx2[:, T], Gs[c][:, :, 2], 0.5, Gs[c][:, :, 0], Op.mult, Op.add)
            nc.vector.scalar_tensor_tensor(gy1[:, T], Gs[c][:, :, 3], -0.5, Gs[c][:, :, 1], Op.mult, Op.add)
            nc.vector.scalar_tensor_tensor(gy2[:, T], Gs[c][:, :, 3], 0.5, Gs[c][:, :, 1], Op.mult, Op.add)
        nc.vector.tensor_tensor(e1[:, :], gx2[:, :], gx1[:, :], Op.subtract)
        nc.vector.tensor_tensor(e2[:, :], gy2[:, :], gy1[:, :], Op.subtract)
        nc.vector.tensor_tensor(gta[:, :], e1[:, :], e2[:, :], Op.mult)
        m1 = sbe.tile([128, NT], f32); m2 = sbe.tile([128, NT], f32)
        whx = sbe.tile([128, NT], f32); why = sbe.tile([128, NT], f32)
        nc.vector.tensor_tensor(m1[:, :], gx1[:, :], sx1[:, :], Op.max)
        nc.vector.tensor_tensor(m2[:, :], gx2[:, :], sx2[:, :], Op.min)
        nc.vector.tensor_tensor(whx[:, :], m2[:, :], m1[:, :], Op.subtract)
        nc.scalar.activation(whx[:, :], whx[:, :], AF.Relu)
        nc.vector.tensor_tensor(m1[:, :], gy1[:, :], sy1[:, :], Op.max)
        nc.vector.tensor_tensor(m2[:, :], gy2[:, :], sy2[:, :], Op.min)
        nc.vector.tensor_tensor(why[:, :], m2[:, :], m1[:, :], Op.subtract)
        nc.scalar.activation(why[:, :], why[:, :], AF.Relu)
        inter = sbe.tile([128, NT], f32)
        nc.vector.tensor_tensor(inter[:, :], whx[:, :], why[:, :], Op.mult)
        den = sbe.tile([128, NT], f32)
        nc.vector.tensor_tensor(den[:, :], sa[:, :], gta[:, :], Op.add)
        nc.vector.scalar_tensor_tensor(den[:, :], den[:, :], 1e-7, inter[:, :], Op.add, Op.subtract)
        nc.vector.reciprocal(den[:, :], den[:, :])
        miou = sbe.tile([128, NT], f32)
        nc.vector.tensor_tensor(miou[:, :], inter[:, :], den[:, :], Op.mult)
        nc.vector.tensor_tensor(miou[:, :], miou[:, :], w_all[:, :], Op.mult)

        # box loss: sum |s-t| over 4 coords * miou * w
        bsum = sbe.tile([128, NT], f32)
        bd = sbe.tile([128, NT], f32)
        for col in range(4):
            for c in range(4):
                T = slice(4 * c, 4 * c + 4)
                nc.vector.tensor_tensor(bd[:, T], s_c[:, T, col], Gs[c][:, :, col], Op.subtract)
            nc.scalar.activation(bd[:, :], bd[:, :], AF.Abs)
            if col == 0:
                nc.vector.tensor_copy(bsum[:, :], bd[:, :])
            else:
                nc.vector.tensor_tensor(bsum[:, :], bsum[:, :], bd[:, :], Op.add)
        nc.vector.tensor_tensor(bsum[:, :], bsum[:, :], miou[:, :], Op.mult)

        # conf loss: (s_conf - t_conf*miou)^2 * w   (miou already w-masked)
        cf = sbe.tile([128, NT], f32)
        for c in range(4):
            T = slice(4 * c, 4 * c + 4)
            nc.vector.tensor_tensor(cf[:, T], Gs[c][:, :, 4], miou[:, T], Op.mult)
        nc.vector.tensor_tensor(cf[:, :], s_c[:, :, 4], cf[:, :], Op.subtract)
        nc.vector.tensor_tensor(cf[:, :], cf[:, :], cf[:, :], Op.mult)
        nc.vector.tensor_tensor(cf[:, :], cf[:, :], w_all[:, :], Op.mult)

        # reductions
        acc = sbe.tile([128, 4], f32)
        nc.vector.reduce_sum(acc[:, 0:1], kl[:, :], axis=AX.X)
        nc.vector.reduce_sum(acc[:, 1:2], bsum[:, :], axis=AX.X)
        nc.vector.reduce_sum(acc[:, 2:3], cf[:, :], axis=AX.X)
        nc.vector.reduce_sum(acc[:, 3:4], w_all[:, :], axis=AX.X)
        accp_full = ps.tile([1, 512], f32, tag="ps_acc", name="accp")
        accrow = accp_full[0:1, 0:4]
        nc.tensor.matmul(accrow[0:1, :], c_ones_col[:, 0:1], acc[:, :])
        res = sbe.tile([1, 8], f32)
        nc.vector.memset(res[:1, :], 0.0)
        nc.vector.tensor_copy(res[:1, 0:4], accrow[0:1, 0:4])
        Msafe = sbe.tile([1, 1], f32, tag="msafe")
        nc.vector.tensor_scalar(Msafe[:1, :], res[:1, 3:4], 1.0, None, Op.max)
        nc.vector.reciprocal(Msafe[:1, :], Msafe[:1, :])
        nc.vector.tensor_scalar(res[:1, 4:5], Msafe[:1, :], 1.0, None, Op.mult)
        nc.sync.dma_start(out.ap()[:, :], res[:1, :])

    nc.compile()
    return nc


def _consts():
    f32 = np.float32
    if "consts" not in _CACHE:
        cpack = np.zeros((128, 258), f32)
        cpack[:, 0:128] = np.tril(np.ones((128, 128), f32), -1)
        cpack[:, 128:256] = np.eye(128, dtype=f32)
        cpack[:, 256] = np.arange(128, dtype=f32) + 1.0
        cpack[:, 257] = 1.0
        _CACHE["consts"] = {
            "cpack": cpack,
            "ones_col": np.ones((1, 128), f32),
            "kbig_lhs": np.full((128, 128), KILLV, np.float16),
        }
    return _CACHE["consts"]


def _prep_core_inputs(s_img, t_img):
    """Build per-core inputs from one (padded) student image [2048, 85] and
    the ORIGINAL teacher rows (1024 or 2048, uncompacted)."""
    f32 = np.float32
    W = _CACHE["W"]
    s = np.asarray(s_img, f32)
    t = np.asarray(t_img, f32)

    # teacher compaction (order-preserving), reference conf>0.5 + fallback
    mask = t[:, 4] > 0.5
    if not mask.any():
        mask = np.zeros_like(mask)
        mask[int(np.argmax(t[:, 4]))] = True
    vidx = np.where(mask)[0]
    nv = len(vidx)
    assert nv <= W, f"valid teachers {nv} > W={W}"
    tc = t[vidx]

    tx1 = tc[:, 0] - tc[:, 2] / f32(2); tx2 = tc[:, 0] + tc[:, 2] / f32(2)
    ty1 = tc[:, 1] - tc[:, 3] / f32(2); ty2 = tc[:, 1] + tc[:, 3] / f32(2)
    ta = ((tx2 - tx1) * (ty2 - ty1)).astype(f32)

    tcoords = np.zeros((5, W), np.float16)
    tcoords[0, :nv] = tx1; tcoords[1, :nv] = tx2
    tcoords[2, :nv] = ty1; tcoords[3, :nv] = ty2
    tcoords[4, :] = np.arange(W, dtype=f32)
    tcoords_rep = np.broadcast_to(tcoords[None, :, :], (128, 5, W)).copy()

    ta_row = np.full((1, W), 4.0, f32)
    ta_row[0, :nv] = ta
    inv_row = np.zeros((1, W), np.float16)
    inv_row[0, nv:] = 1.0

    t_rows_nat = np.zeros((W, D), f32)
    t_rows_nat[:nv] = tc

    s_cols = np.empty((128, NT, 5), f32)
    s_logits = np.empty((128, NT, 80), f32)
    for j in range(NT):
        s_cols[:, j, :] = s[j*128:(j+1)*128, :5]
        s_logits[:, j, :] = s[j*128:(j+1)*128, 5:]

    return {
        "tcoords": tcoords_rep, "ta_row": ta_row, "inv_row": inv_row,
        "s_cols": s_cols, "s_logits": s_logits,
        "t_rows_nat": t_rows_nat, **_consts(),
    }, vidx


def _pad_scale1(s):
    """Pad students [1024, 85] -> [2048, 85] with inert rows."""
    f32 = np.float32
    ns = np.zeros((NS, D), f32)
    ns[:s.shape[0]] = s
    ns[s.shape[0]:, 0] = PAD_X
    ns[s.shape[0]:, 2] = 1.0
    ns[s.shape[0]:, 3] = 1.0
    return ns


def _max_nv(*teachers):
    best = 1
    for t in teachers:
        for b in range(t.shape[0]):
            best = max(best, int((t[b, :, 4] > 0.5).sum()))
    return best


def kernel(student_out0, teacher_out0, student_out1, teacher_out1):
    from concourse.bass_utils import run_bass_kernel_spmd

    student_out0 = np.asarray(student_out0, np.float32)
    teacher_out0 = np.asarray(teacher_out0, np.float32)
    student_out1 = np.asarray(student_out1, np.float32)
    teacher_out1 = np.asarray(teacher_out1, np.float32)

    W = (_max_nv(teacher_out0, teacher_out1) + 63) // 64 * 64
    if _CACHE.get("W") != W:
        _CACHE["W"] = W
        _CACHE["nc"] = _build_nc(W)
    nc = _CACHE["nc"]

    in_maps = []
    for c in range(4):
        m, _ = _prep_core_inputs(student_out0[c], teacher_out0[c])
        in_maps.append(m)
    for c in range(4):
        m, _ = _prep_core_inputs(_pad_scale1(student_out1[c]), teacher_out1[c])
        in_maps.append(m)

    res = run_bass_kernel_spmd(nc, in_maps, core_ids=list(range(8)))

    cls_t = box_t = conf_t = nm = np.float32(0.0)
    for c in range(8):
        o = res.results[c]["out"][0]
        kl_s, box_s, conf_s, M, minv = o[0], o[1], o[2], o[3], o[4]
        cls_t += np.float32(kl_s) * np.float32(minv) * np.float32(TEMP * TEMP)
        box_t += np.float32(box_s) * np.float32(minv) / np.float32(4.0)
        conf_t += np.float32(conf_s) * np.float32(minv)
        nm += np.float32(M)
    nms = max(nm, np.float32(1.0))
    cls_t, box_t, conf_t = cls_t / nms, box_t / nms, conf_t / nms
    total = np.float32(ALPHA) * cls_t + np.float32(BETA) * box_t + np.float32(1.0 - ALPHA - BETA) * conf_t
    return np.float32(total)


# revision 10
# speedup vs baseline: 1.1601x; 1.1601x over previous
"""CrossKD loss kernel for Trainium2, 8 NeuronCores — v4.

Sharding: one (image, scale) pair per core. Cores 0-3: scale-0 images
(2048 anchors); cores 4-7: scale-1 images (1024 anchors) padded to 2048
students with inert rows. One SPMD program on all 8 cores.

v4 changes vs v3 (239us):
  * Used-mask folded into the Ln denominator: a PSUM tensor
    Uta = ta + 65504*(#kills) is maintained by the PE kill-matmuls, and
    lnS = Ln(Uta + sa) de-ranks killed columns (ln(65504) ~ 11 pushes
    them far below the -1.1 match threshold, and they can never pass it
    since lni <= ~0.8). This deletes both the per-stage U16 scalar copy
    and the full-width av add on vector.
  * W dynamic: ceil64(max valid teachers) instead of hardcoded 1152.
  * Teacher coord rows (tx1,tx2,ty1,ty2,iota) shipped pre-replicated
    [128,5,W] f16 over DMA — the startup PE/scalar replicate cascade is
    gone; only ta + invalid-mask enter via K=1 matmuls (Uta init).
  * x-axis intersection via Scalar relu-form: whx = relu(sw - a - b),
    a = relu(tx1-sx1), b = relu(sx2-tx2) — two TS + one TT leave the
    (bottleneck) vector engine for the (slack) scalar engine.
  * KL loss computed in 4-tile chunks pumped as fill work inside the
    stage loop (chunks 0-2); only the last chunk runs in the epilogue.
Host: sums the 4 accumulators over 8 cores, normalizes, weighted sum.
"""
import numpy as np

ALPHA, BETA, TEMP = 0.6, 0.3, 4.0
LN_THR = -1.0986122886681098   # iou > 0.5  <=>  r > 1/3  <=>  ln r > ln(1/3)
KILLV = 65504.0                # f16 max; ln(ta+KILLV+sa) ~ 11.1 >> |LN_THR|
NS = 2048                # padded students per core
NT = 16                  # student tiles
D = 85

PAD_X = 30000.0          # inert-student x center (fp16-safe)

_CACHE = {}


def _build_nc(W):
    import concourse.bacc as bacc
    import concourse.mybir as mybir
    from concourse.tile import TileContext
    from concourse.alu_op_type import AluOpType as Op
    dt = mybir.dt
    AF = mybir.ActivationFunctionType
    AX = mybir.AxisListType
    f32 = dt.float32
    f16 = dt.float16

    nc = bacc.Bacc("TRN2", num_devices=8, debug=False)

    # ---- DRAM I/O ----
    tcoords = nc.dram_tensor("tcoords", [128, 5, W], f16, kind="ExternalInput")  # tx1,tx2,ty1,ty2,iota
    ta_row_d = nc.dram_tensor("ta_row", [1, W], f32, kind="ExternalInput")
    inv_row_d = nc.dram_tensor("inv_row", [1, W], f16, kind="ExternalInput")
    s_cols = nc.dram_tensor("s_cols", [128, NT, 5], f32, kind="ExternalInput")
    s_logits = nc.dram_tensor("s_logits", [128, NT, 80], f32, kind="ExternalInput")
    t_rows_nat = nc.dram_tensor("t_rows_nat", [W, D], f32, kind="ExternalInput")
    p1col = nc.dram_tensor("p1col", [128, 1], f32, kind="ExternalInput")      # p+1
    ltmask = nc.dram_tensor("ltmask", [128, 128], f32, kind="ExternalInput")  # strict lower tri
    identity = nc.dram_tensor("identity", [128, 128], f32, kind="ExternalInput")
    ones_col = nc.dram_tensor("ones_col", [1, 128], f32, kind="ExternalInput")
    ones128_col = nc.dram_tensor("ones128_col", [128, 1], f32, kind="ExternalInput")
    kbig_lhs = nc.dram_tensor("kbig_lhs", [128, 128], f16, kind="ExternalInput")  # 65504
    out = nc.dram_tensor("out", [1, 8], f32, kind="ExternalOutput")

    # PSUM-bank-aligned accumulation chunks
    CH = []
    o = 0
    while o < W:
        n = min(512, W - o)
        CH.append((o, n))
        o += n

    from contextlib import ExitStack
    with TileContext(nc) as tc, ExitStack() as stack:
        # Load the one activation table that serves every func used here
        # (exp, ln, relu, copy, abs) so the table-load pass never swaps.
        from concourse.hw_specs import get_activation_tables
        _tabs = list(get_activation_tables(nc.m.arch))
        nc.scalar.add_instruction(mybir.InstLoadActFuncSet(
            name=nc.scalar.bass.get_next_instruction_name(),
            act_func_set_id=_tabs.index("natural_log_exp_and_others")))
        sb = stack.enter_context(tc.tile_pool(name="sbp", bufs=1))
        ps = stack.enter_context(tc.tile_pool(name="ps", bufs=1, space="PSUM"))
        sbb = stack.enter_context(tc.tile_pool(name="sbb", bufs=2))
        sbr = stack.enter_context(tc.tile_pool(name="sbr", bufs=3))
        sbit = stack.enter_context(tc.tile_pool(name="sbit", bufs=2))

        # ---------- inputs (critical-path first; big tco on the scalar queue) ----------
        tco = sb.tile([128, 5, W], f16)
        nc.scalar.dma_start(tco[:, :, :], tcoords.ap()[:, :, :])
        s_c = sb.tile([128, NT, 5], f32)
        nc.scalar.dma_start(s_c[:, :, :], s_cols.ap()[:, :, :])
        c_ones1 = sb.tile([1, 128], f32); nc.sync.dma_start(c_ones1[:, :], ones_col.ap()[:, :])
        ta_row = sb.tile([1, W], f32)
        nc.sync.dma_start(ta_row[:1, :], ta_row_d.ap()[:, :])
        inv_row = sb.tile([1, W], f16)
        nc.sync.dma_start(inv_row[:1, :], inv_row_d.ap()[:, :])
        c_kbig = sb.tile([128, 128], f16); nc.sync.dma_start(c_kbig[:, :], kbig_lhs.ap()[:, :])
        c_p1 = sb.tile([128, 1], f32); nc.sync.dma_start(c_p1[:, :], p1col.ap()[:, :])
        c_lt = sb.tile([128, 128], f32); nc.sync.dma_start(c_lt[:, :], ltmask.ap()[:, :])
        c_id = sb.tile([128, 128], f32); nc.sync.dma_start(c_id[:, :], identity.ap()[:, :])
        c_ones_col = sb.tile([128, 1], f32); nc.sync.dma_start(c_ones_col[:, :], ones128_col.ap()[:, :])
        slg = sb.tile([128, NT, 80], f32)
        nc.sync.dma_start(slg[:, :, :], s_logits.ap()[:, :, :])

        # ---------- Uta init: PSUM = ta + KILLV*invalid ----------
        Uta = ps.tile([128, W], f32, tag="ps_U", name="Uta")
        for (o, n) in CH:
            nc.tensor.matmul(Uta[:, o:o+n], c_ones1[:1, :], ta_row[0:1, o:o+n],
                             start=True, stop=True, skip_group_check=True)
        for (o, n) in CH:
            nc.tensor.matmul(Uta[:, o:o+n], c_kbig[0:1, :], inv_row[:1, o:o+n],
                             start=False, stop=True, skip_group_check=True)

        # ---------- student scalars [128, NT] ----------
        sx1 = sb.tile([128, NT], f32); sx2 = sb.tile([128, NT], f32)
        sy1 = sb.tile([128, NT], f32); sy2 = sb.tile([128, NT], f32)
        sa = sb.tile([128, NT], f32)
        nc.vector.scalar_tensor_tensor(sx1[:, :], s_c[:, :, 2], -0.5, s_c[:, :, 0], Op.mult, Op.add)
        nc.vector.scalar_tensor_tensor(sx2[:, :], s_c[:, :, 2], 0.5, s_c[:, :, 0], Op.mult, Op.add)
        nc.vector.scalar_tensor_tensor(sy1[:, :], s_c[:, :, 3], -0.5, s_c[:, :, 1], Op.mult, Op.add)
        nc.vector.scalar_tensor_tensor(sy2[:, :], s_c[:, :, 3], 0.5, s_c[:, :, 1], Op.mult, Op.add)
        nsx1 = sb.tile([128, NT], f32)
        nc.vector.tensor_scalar(nsx1[:, :], sx1[:, :], -1.0, None, Op.mult)
        tmpw = sb.tile([128, NT], f32)
        nc.vector.tensor_tensor(sa[:, :], sx2[:, :], sx1[:, :], Op.subtract)
        nc.vector.tensor_tensor(tmpw[:, :], sy2[:, :], sy1[:, :], Op.subtract)
        nc.vector.tensor_tensor(sa[:, :], sa[:, :], tmpw[:, :], Op.mult)
        sa1e7 = sb.tile([128, NT], f32)
        nc.vector.tensor_scalar(sa1e7[:, :], sa[:, :], 1e-7, None, Op.add)

        # ---------- per-stage results ----------
        w_all = sb.tile([128, NT], f32)
        Gs = [sb.tile([128, 4, D], f32, tag=f"G{c}", name=f"G{c}") for c in range(4)]
        se_all = sb.tile([128, NT], f32)
        tse_all = sb.tile([128, NT], f32)
        klA = sb.tile([128, NT], f32)
        klB = sb.tile([128, NT], f32)

        # ---------- production of lni tiles (fill work) ----------
        prod = {}

        def make_prod(j):
            st = {}

            def p_a():
                st["a"] = sbb.tile([128, W], f16, tag="pa", name=f"a{j}")
                nc.scalar.activation(st["a"][:, :], tco[:, 0, :], AF.Relu,
                                     bias=nsx1[:, j:j+1])

            def p_b():
                st["b"] = sbb.tile([128, W], f16, tag="pb", name=f"b{j}")
                nc.scalar.activation(st["b"][:, :], tco[:, 1, :], AF.Relu,
                                     scale=-1.0, bias=sx2[:, j:j+1])

            def p_cx():
                st["cx"] = sbb.tile([128, W], f16, tag="pcx", name=f"cx{j}")
                nc.vector.tensor_tensor(st["cx"][:, :], st["a"][:, :], st["b"][:, :], Op.add)

            def p_whx():
                st["whx"] = sbb.tile([128, W], f16, tag="pwhx", name=f"whx{j}")
                nc.scalar.activation(st["whx"][:, :], st["cx"][:, :], AF.Relu,
                                     scale=-1.0, bias=s_c[:, j, 2:3])

            def p_m1y():
                st["m1y"] = sbb.tile([128, W], f16, tag="pm1y", name=f"m1y{j}")
                nc.vector.tensor_scalar(st["m1y"][:, :], tco[:, 2, :], sy1[:, j:j+1], None, Op.max)

            def p_t1y():
                st["t1y"] = sbb.tile([128, W], f16, tag="pt1y", name=f"t1y{j}")
                nc.vector.tensor_scalar(st["t1y"][:, :], tco[:, 3, :], sy2[:, j:j+1], None, Op.min)

            def p_wyr():
                st["wyr"] = sbb.tile([128, W], f16, tag="pwyr", name=f"wyr{j}")
                nc.vector.tensor_tensor(st["wyr"][:, :], st["t1y"][:, :], st["m1y"][:, :], Op.subtract)

            def p_why():
                st["why"] = sbb.tile([128, W], f16, tag="pwhy", name=f"why{j}")
                nc.scalar.activation(st["why"][:, :], st["wyr"][:, :], AF.Relu)

            def p_inter():
                st["inter"] = sbb.tile([128, W], f16, tag="pinter", name=f"inter{j}")
                nc.vector.tensor_tensor(st["inter"][:, :], st["whx"][:, :], st["why"][:, :], Op.mult)

            def p_lni():
                prod[j] = sbr.tile([128, W], f16, tag="lni", name=f"lni{j}")
                nc.scalar.activation(prod[j][:, :], st["inter"][:, :], AF.Ln)

            return [p_a, p_b, p_m1y, p_t1y, p_cx, p_wyr, p_whx, p_why, p_inter, p_lni]

        # ---------- KL chunk closures (fill work; chunk c = tiles 4c..4c+3) ----------
        def make_kl(c):
            T = slice(4 * c, 4 * c + 4)
            st = {}

            def k_sexp():
                st["sex"] = sbb.tile([128, 4, 80], f32, tag="ksex", name=f"sex{c}")
                nc.scalar.activation(st["sex"][:, :, :], slg[:, T, :], AF.Exp, scale=1.0 / TEMP)

            def k_texp():
                st["tex"] = sbb.tile([128, 4, 80], f32, tag="ktex", name=f"tex{c}")
                nc.scalar.activation(st["tex"][:, :, :], Gs[c][:, :, 5:], AF.Exp, scale=1.0 / TEMP)

            def k_se():
                nc.vector.tensor_reduce(se_all[:, T], st["sex"][:, :, :], AX.X, Op.add)

            def k_tse():
                nc.vector.tensor_reduce(tse_all[:, T], st["tex"][:, :, :], AX.X, Op.add)

            def k_pa():
                st["pA"] = sbb.tile([128, 4, 80], f32, tag="kpA", name=f"pA{c}")
                nc.vector.tensor_tensor(st["pA"][:, :, :], st["tex"][:, :, :], Gs[c][:, :, 5:], Op.mult)

            def k_ka():
                nc.vector.tensor_reduce(klA[:, T], st["pA"][:, :, :], AX.X, Op.add)

            def k_pb():
                st["pB"] = sbb.tile([128, 4, 80], f32, tag="kpB", name=f"pB{c}")
                nc.vector.tensor_tensor(st["pB"][:, :, :], st["tex"][:, :, :], slg[:, T, :], Op.mult)

            def k_kb():
                nc.vector.tensor_reduce(klB[:, T], st["pB"][:, :, :], AX.X, Op.add)

            return [k_sexp, k_texp, k_se, k_tse, k_pa, k_ka, k_pb, k_kb]

        # ---------- work queue ----------
        from collections import deque
        work = deque()

        def pump(n):
            for _ in range(n):
                if work:
                    work.popleft()()
                else:
                    return

        def pump_all():
            while work:
                work.popleft()()

        # prime: production 0 fully, production 1 queued
        for fn in make_prod(0):
            fn()
        work.extend(make_prod(1))
        pump(4)

        # lnS_0 / t_0
        lnS_cur = {}
        t_cur = {}
        lnS_cur[0] = sbb.tile([128, W], f16, tag="lnS", name="lnS0")
        nc.scalar.activation(lnS_cur[0][:, :], Uta[:, :], AF.Ln, bias=sa1e7[:, 0:1])
        t_cur[0] = sbr.tile([128, W], f16, tag="tt", name="t0")
        nc.vector.tensor_tensor(t_cur[0][:, :], prod[0][:, :], lnS_cur[0][:, :], Op.subtract)

        import concourse.bass as bass_mod
        KL_AT = {5: 0, 9: 1, 13: 2}   # stage -> chunk emitted as fill

        # ---------- stages ----------
        for j in range(NT):
            if j + 2 < NT:
                work.extend(make_prod(j + 2))
            if j in KL_AT:
                work.extend(make_kl(KL_AT[j]))

            t_j = t_cur.pop(j)
            top8v = sbit.tile([128, 8], f16, tag="st_top8v")
            nc.vector.max(top8v[:, :], t_j[:, :])
            pos8 = sbit.tile([128, 8], mybir.dt.uint32, tag="st_pos8")
            nc.vector.max_index(pos8[:, :], top8v[:, :], t_j[:, :])

            tid = sbit.tile([128, 1], f32, tag="st_tid")
            nc.vector.tensor_copy(tid[:, :], pos8[:, 0:1])
            act = sbit.tile([128, 1], f32, tag="st_act")
            nc.vector.tensor_scalar(act[:, :], top8v[:, 0:1], float(LN_THR), None, Op.is_gt)
            te1 = sbit.tile([128, 1], f32, tag="st_te1")
            nc.vector.scalar_tensor_tensor(te1[:, :], tid[:, :], c_p1[:, 0:1], act[:, :], Op.add, Op.mult)
            tid_eff = sbit.tile([128, 1], f32, tag="st_tideff")
            nc.vector.tensor_scalar(tid_eff[:, :], te1[:, :], c_p1[:, 0:1], None, Op.subtract)
            # one-hot over W of this stage's proposals: every act=1 proposal's
            # teacher is accepted by its earliest proposer, so proposals and
            # accepts kill the same columns — Uta commits before the conflict
            # round trip resolves.
            # PE: proposal broadcast (transpose) + Uta kill commit + conflict matrix
            tp = ps.tile([128, 128], f32, tag="ps_tp", name="ittp")
            nc.tensor.transpose(tp[0:1, 0:128], tid_eff[:, 0:1], c_id[:, :])
            if j + 1 < NT:
                ohw = sbit.tile([128, W], f16, tag="st_ohw")
                nc.vector.tensor_scalar(ohw[:, :], tco[:, 4, :], tid_eff[:, 0:1], None, Op.is_equal)
                for (o, n) in CH:
                    nc.tensor.matmul(Uta[:, o:o+n], c_kbig[:, :], ohw[:, o:o+n],
                                     start=False, stop=True, skip_group_check=True)

            if j + 1 < NT:
                lnS_cur[j + 1] = sbb.tile([128, W], f16, tag="lnS", name=f"lnS{j+1}")
                nc.scalar.activation(lnS_cur[j + 1][:, :], Uta[:, :], AF.Ln, bias=sa1e7[:, j+1:j+2])

            itrow = sbit.tile([1, 128], f32, tag="st_itrow")
            nc.scalar.copy(itrow[:1, :], tp[0:1, 0:128])
            trep = ps.tile([128, 128], f32, tag="ps_trep", name="ittrep")
            nc.tensor.matmul(trep[:, :], c_ones1[:1, :], itrow[:1, :])

            if j + 1 < NT:
                # drain fill until lni_{j+1} is emitted, then chain t_{j+1}
                while work and (j + 1) not in prod:
                    work.popleft()()
                t_cur[j + 1] = sbr.tile([128, W], f16, tag="tt", name=f"t{j+1}")
                nc.vector.tensor_tensor(t_cur[j + 1][:, :], prod.pop(j + 1)[:, :],
                                        lnS_cur.pop(j + 1)[:, :], Op.subtract)

            # conflict: an earlier partition proposes the same teacher -> lost
            cnt = sbit.tile([128, 1], f32, tag="st_cnt")
            escr = sbit.tile([128, 128], f32, tag="st_escr")
            nc.vector.scalar_tensor_tensor(escr[:, :], trep[:, :], tid_eff[:, 0:1], c_lt[:, :],
                                           Op.is_equal, Op.mult, accum_out=cnt[:, 0:1])
            notlost = sbit.tile([128, 1], f32, tag="st_nl")
            nc.vector.tensor_scalar(notlost[:, :], cnt[:, :], 0.5, None, Op.is_le)
            nc.vector.tensor_tensor(w_all[:, j:j+1], act[:, :], notlost[:, :], Op.mult)
            tsp1 = sbit.tile([128, 1], f32, tag="st_tsp1")
            nc.vector.scalar_tensor_tensor(tsp1[:, :], tid[:, :], 1.0, w_all[:, j:j+1], Op.add, Op.mult)
            tid_sel = sbit.tile([128, 1], f32, tag="st_tidsel")
            nc.vector.tensor_scalar(tid_sel[:, :], tsp1[:, :], 1.0, None, Op.subtract)

            # gather matched teacher rows from DRAM by index (idle DMA engines)
            tidc = sbit.tile([128, 1], f32, tag="st_tidc")
            nc.gpsimd.tensor_scalar(tidc[:, :], tid_sel[:, :], 0.0, None, Op.max)
            tidi = sbit.tile([128, 1], mybir.dt.int32, tag="st_tidi")
            nc.gpsimd.tensor_copy(tidi[:, :], tidc[:, :])
            nc.gpsimd.indirect_dma_start(
                out=Gs[j // 4][:, j % 4, :], out_offset=None,
                in_=t_rows_nat.ap()[:, :],
                in_offset=bass_mod.IndirectOffsetOnAxis(ap=tidi[:, 0:1], axis=0),
            )

            if j + 1 < NT:
                pump(3)  # keep KL-chunk backlog from piling onto the chain
            else:
                pump_all()

        pump_all()

        # ---------- epilogue ----------
        sbe = stack.enter_context(tc.tile_pool(name="sbe", bufs=1))

        # last KL chunk
        for fn in make_kl(3):
            fn()

        # klD = ln se - ln tse
        lnse = sbe.tile([128, NT], f32)
        nc.scalar.activation(lnse[:, :], se_all[:, :], AF.Ln)
        lntse = sbe.tile([128, NT], f32)
        nc.scalar.activation(lntse[:, :], tse_all[:, :], AF.Ln)
        klD = sbe.tile([128, NT], f32)
        nc.vector.tensor_tensor(klD[:, :], lnse[:, :], lntse[:, :], Op.subtract)

        # kl = 0.25*(klA - klB)/tse + klD, weighted by w
        kl = sbe.tile([128, NT], f32)
        nc.vector.tensor_tensor(kl[:, :], klA[:, :], klB[:, :], Op.subtract)
        rtse = sbe.tile([128, NT], f32)
        nc.vector.reciprocal(rtse[:, :], tse_all[:, :])
        nc.vector.tensor_scalar(rtse[:, :], rtse[:, :], 1.0 / TEMP, None, Op.mult)
        nc.vector.tensor_tensor(kl[:, :], kl[:, :], rtse[:, :], Op.mult)
        nc.vector.tensor_tensor(kl[:, :], kl[:, :], klD[:, :], Op.add)
        nc.vector.tensor_tensor(kl[:, :], kl[:, :], w_all[:, :], Op.mult)

        # --- exact miou recompute + box/conf (chunked over the 4 G tiles) ---
        gx1 = sbe.tile([128, NT], f32); gx2 = sbe.tile([128, NT], f32)
        gy1 = sbe.tile([128, NT], f32); gy2 = sbe.tile([128, NT], f32)
        gta = sbe.tile([128, NT], f32)
        e1 = sbe.tile([128, NT], f32); e2 = sbe.tile([128, NT], f32)
        for c in range(4):
            T = slice(4 * c, 4 * c + 4)
            nc.vector.scalar_tensor_tensor(gx1[:, T], Gs[c][:, :, 2], -0.5, Gs[c][:, :, 0], Op.mult, Op.add)
            nc.vector.scalar_tensor_tensor(gx2[:, T], Gs[c][:, :, 2], 0.5, Gs[c][:, :, 0], Op.mult, Op.add)
            nc.vector.scalar_tensor_tensor(gy1[:, T], Gs[c][:, :, 3], -0.5, Gs[c][:, :, 1], Op.mult, Op.add)
            nc.vector.scalar_tensor_tensor(gy2[:, T], Gs[c][:, :, 3], 0.5, Gs[c][:, :, 1], Op.mult, Op.add)
        nc.vector.tensor_tensor(e1[:, :], gx2[:, :], gx1[:, :], Op.subtract)
        nc.vector.tensor_tensor(e2[:, :], gy2[:, :], gy1[:, :], Op.subtract)
        nc.vector.tensor_tensor(gta[:, :], e1[:, :], e2[:, :], Op.mult)
        m1 = sbe.tile([128, NT], f32); m2 = sbe.tile([128, NT], f32)
        whx = sbe.tile([128, NT], f32); why = sbe.tile([128, NT], f32)
        nc.vector.tensor_tensor(m1[:, :], gx1[:, :], sx1[:, :], Op.max)
        nc.vector.tensor_tensor(m2[:, :], gx2[:, :], sx2[:, :], Op.min)
        nc.vector.tensor_tensor(whx[:, :], m2[:, :], m1[:, :], Op.subtract)
        nc.scalar.activation(whx[:, :], whx[:, :], AF.Relu)
        nc.vector.tensor_tensor(m1[:, :], gy1[:, :], sy1[:, :], Op.max)
        nc.vector.tensor_tensor(m2[:, :], gy2[:, :], sy2[:, :], Op.min)
        nc.vector.tensor_tensor(why[:, :], m2[:, :], m1[:, :], Op.subtract)
        nc.scalar.activation(why[:, :], why[:, :], AF.Relu)
        inter = sbe.tile([128, NT], f32)
        nc.vector.tensor_tensor(inter[:, :], whx[:, :], why[:, :], Op.mult)
        den = sbe.tile([128, NT], f32)
        nc.vector.tensor_tensor(den[:, :], sa[:, :], gta[:, :], Op.add)
        nc.vector.scalar_tensor_tensor(den[:, :], den[:, :], 1e-7, inter[:, :], Op.add, Op.subtract)
        nc.vector.reciprocal(den[:, :], den[:, :])
        miou = sbe.tile([128, NT], f32)
        nc.vector.tensor_tensor(miou[:, :], inter[:, :], den[:, :], Op.mult)
        nc.vector.tensor_tensor(miou[:, :], miou[:, :], w_all[:, :], Op.mult)

        # box loss: sum |s-t| over 4 coords * miou * w
        bsum = sbe.tile([128, NT], f32)
        bd = sbe.tile([128, NT], f32)
        for col in range(4):
            for c in range(4):
                T = slice(4 * c, 4 * c + 4)
                nc.vector.tensor_tensor(bd[:, T], s_c[:, T, col], Gs[c][:, :, col], Op.subtract)
            nc.scalar.activation(bd[:, :], bd[:, :], AF.Abs)
            if col == 0:
                nc.vector.tensor_copy(bsum[:, :], bd[:, :])
            else:
                nc.vector.tensor_tensor(bsum[:, :], bsum[:, :], bd[:, :], Op.add)
        nc.vector.tensor_tensor(bsum[:, :], bsum[:, :], miou[:, :], Op.mult)

        # conf loss: (s_conf - t_conf*miou)^2 * w   (miou already w-masked)
        cf = sbe.tile([128, NT], f32)
        for c in range(4):
            T = slice(4 * c, 4 * c + 4)
            nc.vector.tensor_tensor(cf[:, T], Gs[c][:, :, 4], miou[:, T], Op.mult)
        nc.vector.tensor_tensor(cf[:, :], s_c[:, :, 4], cf[:, :], Op.subtract)
        nc.vector.tensor_tensor(cf[:, :], cf[:, :], cf[:, :], Op.mult)
        nc.vector.tensor_tensor(cf[:, :], cf[:, :], w_all[:, :], Op.mult)

        # reductions
        acc = sbe.tile([128, 4], f32)
        nc.vector.reduce_sum(acc[:, 0:1], kl[:, :], axis=AX.X)
        nc.vector.reduce_sum(acc[:, 1:2], bsum[:, :], axis=AX.X)
        nc.vector.reduce_sum(acc[:, 2:3], cf[:, :], axis=AX.X)
        nc.vector.reduce_sum(acc[:, 3:4], w_all[:, :], axis=AX.X)
        accp_full = ps.tile([1, 512], f32, tag="ps_acc", name="accp")
        accrow = accp_full[0:1, 0:4]
        nc.tensor.matmul(accrow[0:1, :], c_ones_col[:, 0:1], acc[:, :])
        res = sbe.tile([1, 8], f32)
        nc.vector.memset(res[:1, :], 0.0)
        nc.vector.tensor_copy(res[:1, 0:4], accrow[0:1, 0:4])
        Msafe = sbe.tile([1, 1], f32, tag="msafe")
        nc.vector.tensor_scalar(Msafe[:1, :], res[:1, 3:4], 1.0, None, Op.max)
        nc.vector.reciprocal(Msafe[:1, :], Msafe[:1, :])
        nc.vector.tensor_scalar(res[:1, 4:5], Msafe[:1, :], 1.0, None, Op.mult)
        nc.sync.dma_start(out.ap()[:, :], res[:1, :])

    nc.compile()
    return nc


def _consts():
    f32 = np.float32
    if "consts" not in _CACHE:
        _CACHE["consts"] = {
            "p1col": (np.arange(128, dtype=f32)[:, None] + 1.0),
            "ltmask": np.tril(np.ones((128, 128), f32), -1),
            "identity": np.eye(128, dtype=f32),
            "ones_col": np.ones((1, 128), f32),
            "ones128_col": np.ones((128, 1), f32),
            "kbig_lhs": np.full((128, 128), KILLV, np.float16),
        }
    return _CACHE["consts"]


def _prep_core_inputs(s_img, t_img):
    """Build per-core inputs from one (padded) student image [2048, 85] and
    the ORIGINAL teacher rows (1024 or 2048, uncompacted)."""
    f32 = np.float32
    W = _CACHE["W"]
    s = np.asarray(s_img, f32)
    t = np.asarray(t_img, f32)

    # teacher compaction (order-preserving), reference conf>0.5 + fallback
    mask = t[:, 4] > 0.5
    if not mask.any():
        mask = np.zeros_like(mask)
        mask[int(np.argmax(t[:, 4]))] = True
    vidx = np.where(mask)[0]
    nv = len(vidx)
    assert nv <= W, f"valid teachers {nv} > W={W}"
    tc = t[vidx]

    tx1 = tc[:, 0] - tc[:, 2] / f32(2); tx2 = tc[:, 0] + tc[:, 2] / f32(2)
    ty1 = tc[:, 1] - tc[:, 3] / f32(2); ty2 = tc[:, 1] + tc[:, 3] / f32(2)
    ta = ((tx2 - tx1) * (ty2 - ty1)).astype(f32)

    tcoords = np.zeros((5, W), np.float16)
    tcoords[0, :nv] = tx1; tcoords[1, :nv] = tx2
    tcoords[2, :nv] = ty1; tcoords[3, :nv] = ty2
    tcoords[4, :] = np.arange(W, dtype=f32)
    tcoords_rep = np.broadcast_to(tcoords[None, :, :], (128, 5, W)).copy()

    ta_row = np.full((1, W), 4.0, f32)
    ta_row[0, :nv] = ta
    inv_row = np.zeros((1, W), np.float16)
    inv_row[0, nv:] = 1.0

    t_rows_nat = np.zeros((W, D), f32)
    t_rows_nat[:nv] = tc

    s_cols = np.empty((128, NT, 5), f32)
    s_logits = np.empty((128, NT, 80), f32)
    for j in range(NT):
        s_cols[:, j, :] = s[j*128:(j+1)*128, :5]
        s_logits[:, j, :] = s[j*128:(j+1)*128, 5:]

    return {
        "tcoords": tcoords_rep, "ta_row": ta_row, "inv_row": inv_row,
        "s_cols": s_cols, "s_logits": s_logits,
        "t_rows_nat": t_rows_nat, **_consts(),
    }, vidx


def _pad_scale1(s):
    """Pad students [1024, 85] -> [2048, 85] with inert rows."""
    f32 = np.float32
    ns = np.zeros((NS, D), f32)
    ns[:s.shape[0]] = s
    ns[s.shape[0]:, 0] = PAD_X
    ns[s.shape[0]:, 2] = 1.0
    ns[s.shape[0]:, 3] = 1.0
    return ns


def _max_nv(*teachers):
    best = 1
    for t in teachers:
        for b in range(t.shape[0]):
            best = max(best, int((t[b, :, 4] > 0.5).sum()))
    return best


def kernel(student_out0, teacher_out0, student_out1, teacher_out1):
    from concourse.bass_utils import run_bass_kernel_spmd

    student_out0 = np.asarray(student_out0, np.float32)
    teacher_out0 = np.asarray(teacher_out0, np.float32)
    student_out1 = np.asarray(student_out1, np.float32)
    teacher_out1 = np.asarray(teacher_out1, np.float32)

    W = (_max_nv(teacher_out0, teacher_out1) + 63) // 64 * 64
    if _CACHE.get("W") != W:
        _CACHE["W"] = W
        _CACHE["nc"] = _build_nc(W)
    nc = _CACHE["nc"]

    in_maps = []
    for c in range(4):
        m, _ = _prep_core_inputs(student_out0[c], teacher_out0[c])
        in_maps.append(m)
    for c in range(4):
        m, _ = _prep_core_inputs(_pad_scale1(student_out1[c]), teacher_out1[c])
        in_maps.append(m)

    res = run_bass_kernel_spmd(nc, in_maps, core_ids=list(range(8)))

    cls_t = box_t = conf_t = nm = np.float32(0.0)
    for c in range(8):
        o = res.results[c]["out"][0]
        kl_s, box_s, conf_s, M, minv = o[0], o[1], o[2], o[3], o[4]
        cls_t += np.float32(kl_s) * np.float32(minv) * np.float32(TEMP * TEMP)
        box_t += np.float32(box_s) * np.float32(minv) / np.float32(4.0)
        conf_t += np.float32(conf_s) * np.float32(minv)
        nm += np.float32(M)
    nms = max(nm, np.float32(1.0))
    cls_t, box_t, conf_t = cls_t / nms, box_t / nms, conf_t / nms
    total = np.float32(ALPHA) * cls_t + np.float32(BETA) * box_t + np.float32(1.0 - ALPHA - BETA) * conf_t
    return np.float32(total)


# revision 13
# speedup vs baseline: 1.1762x; 1.0138x over previous
"""CrossKD loss kernel for Trainium2, 8 NeuronCores — v4.

Sharding: one (image, scale) pair per core. Cores 0-3: scale-0 images
(2048 anchors); cores 4-7: scale-1 images (1024 anchors) padded to 2048
students with inert rows. One SPMD program on all 8 cores.

v4 changes vs v3 (239us):
  * Used-mask folded into the Ln denominator: a PSUM tensor
    Uta = ta + 65504*(#kills) is maintained by the PE kill-matmuls, and
    lnS = Ln(Uta + sa) de-ranks killed columns (ln(65504) ~ 11 pushes
    them far below the -1.1 match threshold, and they can never pass it
    since lni <= ~0.8). This deletes both the per-stage U16 scalar copy
    and the full-width av add on vector.
  * W dynamic: ceil64(max valid teachers) instead of hardcoded 1152.
  * Teacher coord rows (tx1,tx2,ty1,ty2,iota) shipped pre-replicated
    [128,5,W] f16 over DMA — the startup PE/scalar replicate cascade is
    gone; only ta + invalid-mask enter via K=1 matmuls (Uta init).
  * x-axis intersection via Scalar relu-form: whx = relu(sw - a - b),
    a = relu(tx1-sx1), b = relu(sx2-tx2) — two TS + one TT leave the
    (bottleneck) vector engine for the (slack) scalar engine.
  * KL loss computed in 4-tile chunks pumped as fill work inside the
    stage loop (chunks 0-2); only the last chunk runs in the epilogue.
Host: sums the 4 accumulators over 8 cores, normalizes, weighted sum.
"""
import numpy as np

ALPHA, BETA, TEMP = 0.6, 0.3, 4.0
LN_THR = -1.0986122886681098   # iou > 0.5  <=>  r > 1/3  <=>  ln r > ln(1/3)
KILLV = 65504.0                # f16 max; ln(ta+KILLV+sa) ~ 11.1 >> |LN_THR|
NS = 2048                # padded students per core
NT = 16                  # student tiles
D = 85

PAD_X = 30000.0          # inert-student x center (fp16-safe)

_CACHE = {}


def _build_nc(W):
    import concourse.bacc as bacc
    import concourse.mybir as mybir
    from concourse.tile import TileContext
    from concourse.alu_op_type import AluOpType as Op
    dt = mybir.dt
    AF = mybir.ActivationFunctionType
    AX = mybir.AxisListType
    f32 = dt.float32
    f16 = dt.float16

    nc = bacc.Bacc("TRN2", num_devices=8, debug=False)

    # ---- DRAM I/O ----
    tcoords = nc.dram_tensor("tcoords", [128, 5, W], f16, kind="ExternalInput")  # tx1,tx2,ty1,ty2,iota
    ta_row_d = nc.dram_tensor("ta_row", [1, W], f32, kind="ExternalInput")
    inv_row_d = nc.dram_tensor("inv_row", [1, W], f16, kind="ExternalInput")
    s_cols = nc.dram_tensor("s_cols", [128, NT, 5], f32, kind="ExternalInput")
    s_pre_d = nc.dram_tensor("s_pre", [128, NT, 7], f32, kind="ExternalInput")  # sx1,sx2,sy1,sy2,nsx1,sa,sa1e7
    s_logits = nc.dram_tensor("s_logits", [128, NT, 80], f32, kind="ExternalInput")
    t_rows_nat = nc.dram_tensor("t_rows_nat", [W, D], f32, kind="ExternalInput")
    p1col = nc.dram_tensor("p1col", [128, 1], f32, kind="ExternalInput")      # p+1
    ltmask = nc.dram_tensor("ltmask", [128, 128], f32, kind="ExternalInput")  # strict lower tri
    identity = nc.dram_tensor("identity", [128, 128], f32, kind="ExternalInput")
    ones_col = nc.dram_tensor("ones_col", [1, 128], f32, kind="ExternalInput")
    ones128_col = nc.dram_tensor("ones128_col", [128, 1], f32, kind="ExternalInput")
    kbig_lhs = nc.dram_tensor("kbig_lhs", [128, 128], f16, kind="ExternalInput")  # 65504
    out = nc.dram_tensor("out", [1, 8], f32, kind="ExternalOutput")

    # PSUM-bank-aligned accumulation chunks
    CH = []
    o = 0
    while o < W:
        n = min(512, W - o)
        CH.append((o, n))
        o += n

    from contextlib import ExitStack
    with TileContext(nc) as tc, ExitStack() as stack:
        # Load the one activation table that serves every func used here
        # (exp, ln, relu, copy, abs) so the table-load pass never swaps.
        from concourse.hw_specs import get_activation_tables
        _tabs = list(get_activation_tables(nc.m.arch))
        nc.scalar.add_instruction(mybir.InstLoadActFuncSet(
            name=nc.scalar.bass.get_next_instruction_name(),
            act_func_set_id=_tabs.index("natural_log_exp_and_others")))
        sb = stack.enter_context(tc.tile_pool(name="sbp", bufs=1))
        ps = stack.enter_context(tc.tile_pool(name="ps", bufs=1, space="PSUM"))
        sbb = stack.enter_context(tc.tile_pool(name="sbb", bufs=2))
        sbr = stack.enter_context(tc.tile_pool(name="sbr", bufs=3))
        sbit = stack.enter_context(tc.tile_pool(name="sbit", bufs=2))
        sbe = stack.enter_context(tc.tile_pool(name="sbe", bufs=1))

        # ---------- inputs (critical-path first; big tco on the scalar queue) ----------
        tco = sb.tile([128, 5, W], f16)
        nc.scalar.dma_start(tco[:, :, :], tcoords.ap()[:, :, :])
        s_pre = sb.tile([128, NT, 7], f32)
        nc.sync.dma_start(s_pre[:, :, :], s_pre_d.ap()[:, :, :])
        s_c = sb.tile([128, NT, 5], f32)
        nc.sync.dma_start(s_c[:, :, :], s_cols.ap()[:, :, :])
        c_ones1 = sb.tile([1, 128], f32); nc.sync.dma_start(c_ones1[:, :], ones_col.ap()[:, :])
        ta_row = sb.tile([1, W], f32)
        nc.sync.dma_start(ta_row[:1, :], ta_row_d.ap()[:, :])
        inv_row = sb.tile([1, W], f16)
        nc.sync.dma_start(inv_row[:1, :], inv_row_d.ap()[:, :])
        c_kbig = sb.tile([128, 128], f16); nc.sync.dma_start(c_kbig[:, :], kbig_lhs.ap()[:, :])
        c_p1 = sb.tile([128, 1], f32); nc.sync.dma_start(c_p1[:, :], p1col.ap()[:, :])
        c_lt = sb.tile([128, 128], f32); nc.sync.dma_start(c_lt[:, :], ltmask.ap()[:, :])
        c_id = sb.tile([128, 128], f32); nc.sync.dma_start(c_id[:, :], identity.ap()[:, :])
        c_ones_col = sb.tile([128, 1], f32); nc.sync.dma_start(c_ones_col[:, :], ones128_col.ap()[:, :])
        slg = sb.tile([128, NT, 80], f32)
        nc.sync.dma_start(slg[:, :, :], s_logits.ap()[:, :, :])

        # ---------- Uta init: PSUM = ta + KILLV*invalid ----------
        Uta = ps.tile([128, W], f32, tag="ps_U", name="Uta")
        for (o, n) in CH:
            nc.tensor.matmul(Uta[:, o:o+n], c_ones1[:1, :], ta_row[0:1, o:o+n],
                             start=True, stop=True, skip_group_check=True)
        for (o, n) in CH:
            nc.tensor.matmul(Uta[:, o:o+n], c_kbig[0:1, :], inv_row[:1, o:o+n],
                             start=False, stop=True, skip_group_check=True)

        # student scalars are host-precomputed planes of s_pre:
        # 0:sx1 1:sx2 2:sy1 3:sy2 4:nsx1 5:sa 6:sa1e7

        # ---------- per-stage results ----------
        w_all = sb.tile([128, NT], f32)
        GS_SPEC = [(0, 4), (4, 4), (8, 4), (12, 3), (15, 1)]
        Gs = [sb.tile([128, n, D], f32, tag=f"G{c}", name=f"G{c}")
              for c, (o, n) in enumerate(GS_SPEC)]

        def g_of(j):
            for c, (o, n) in enumerate(GS_SPEC):
                if o <= j < o + n:
                    return c, j - o
            raise AssertionError(j)
        se_all = sb.tile([128, NT], f32)
        tse_all = sb.tile([128, NT], f32)
        klA = sb.tile([128, NT], f32)
        klB = sb.tile([128, NT], f32)

        # ---------- production of lni tiles (fill work) ----------
        prod = {}

        def make_prod(j):
            st = {}

            def p_a():
                st["a"] = sbb.tile([128, W], f16, tag="pa", name=f"a{j}")
                nc.scalar.activation(st["a"][:, :], tco[:, 0, :], AF.Relu,
                                     bias=s_pre[:, j, 4:5])

            def p_b():
                st["b"] = sbb.tile([128, W], f16, tag="pb", name=f"b{j}")
                nc.scalar.activation(st["b"][:, :], tco[:, 1, :], AF.Relu,
                                     scale=-1.0, bias=s_pre[:, j, 1:2])

            def p_cx():
                st["cx"] = sbb.tile([128, W], f16, tag="pcx", name=f"cx{j}")
                nc.vector.tensor_tensor(st["cx"][:, :], st["a"][:, :], st["b"][:, :], Op.add)

            def p_whx():
                st["whx"] = sbb.tile([128, W], f16, tag="pwhx", name=f"whx{j}")
                nc.scalar.activation(st["whx"][:, :], st["cx"][:, :], AF.Relu,
                                     scale=-1.0, bias=s_c[:, j, 2:3])

            def p_m1y():
                st["m1y"] = sbb.tile([128, W], f16, tag="pm1y", name=f"m1y{j}")
                nc.vector.tensor_scalar(st["m1y"][:, :], tco[:, 2, :], s_pre[:, j, 2:3], None, Op.max)

            def p_t1y():
                st["t1y"] = sbb.tile([128, W], f16, tag="pt1y", name=f"t1y{j}")
                nc.vector.tensor_scalar(st["t1y"][:, :], tco[:, 3, :], s_pre[:, j, 3:4], None, Op.min)

            def p_wyr():
                st["wyr"] = sbb.tile([128, W], f16, tag="pwyr", name=f"wyr{j}")
                nc.vector.tensor_tensor(st["wyr"][:, :], st["t1y"][:, :], st["m1y"][:, :], Op.subtract)

            def p_why():
                st["why"] = sbb.tile([128, W], f16, tag="pwhy", name=f"why{j}")
                nc.scalar.activation(st["why"][:, :], st["wyr"][:, :], AF.Relu)

            def p_inter():
                st["inter"] = sbb.tile([128, W], f16, tag="pinter", name=f"inter{j}")
                nc.vector.tensor_tensor(st["inter"][:, :], st["whx"][:, :], st["why"][:, :], Op.mult)

            def p_lni():
                prod[j] = sbr.tile([128, W], f16, tag="lni", name=f"lni{j}")
                nc.scalar.activation(prod[j][:, :], st["inter"][:, :], AF.Ln)

            return [p_a, p_b, p_m1y, p_t1y, p_cx, p_wyr, p_whx, p_why, p_inter, p_lni]

        # ---------- KL chunk closures (fill work; chunk c = tiles 4c..4c+3) ----------
        def make_kl(c):
            o, nn = GS_SPEC[c]
            T = slice(o, o + nn)
            st = {}

            def k_sexp():
                st["sex"] = sbb.tile([128, 4, 80], f32, tag="ksex", name=f"sex{c}")
                nc.scalar.activation(st["sex"][:, 0:nn, :], slg[:, T, :], AF.Exp, scale=1.0 / TEMP)

            def k_texp():
                st["tex"] = sbb.tile([128, 4, 80], f32, tag="ktex", name=f"tex{c}")
                nc.scalar.activation(st["tex"][:, 0:nn, :], Gs[c][:, :, 5:], AF.Exp, scale=1.0 / TEMP)

            def k_se():
                nc.vector.tensor_reduce(se_all[:, T], st["sex"][:, 0:nn, :], AX.X, Op.add)

            def k_tse():
                nc.vector.tensor_reduce(tse_all[:, T], st["tex"][:, 0:nn, :], AX.X, Op.add)

            def k_pa():
                st["pA"] = sbb.tile([128, 4, 80], f32, tag="kpA", name=f"pA{c}")
                nc.vector.tensor_tensor(st["pA"][:, 0:nn, :], st["tex"][:, 0:nn, :], Gs[c][:, :, 5:], Op.mult)

            def k_ka():
                nc.vector.tensor_reduce(klA[:, T], st["pA"][:, 0:nn, :], AX.X, Op.add)

            def k_pb():
                st["pB"] = sbb.tile([128, 4, 80], f32, tag="kpB", name=f"pB{c}")
                nc.vector.tensor_tensor(st["pB"][:, 0:nn, :], st["tex"][:, 0:nn, :], slg[:, T, :], Op.mult)

            def k_kb():
                nc.vector.tensor_reduce(klB[:, T], st["pB"][:, 0:nn, :], AX.X, Op.add)

            return [k_sexp, k_texp, k_se, k_tse, k_pa, k_ka, k_pb, k_kb]

        # ---------- work queue ----------
        from collections import deque
        work = deque()

        def pump(n):
            for _ in range(n):
                if work:
                    work.popleft()()
                else:
                    return

        def pump_all():
            while work:
                work.popleft()()

        # prime: production 0 fully, production 1 queued
        for fn in make_prod(0):
            fn()
        work.extend(make_prod(1))
        pump(4)

        # lnS_0 / t_0
        lnS_cur = {}
        t_cur = {}
        lnS_cur[0] = sbb.tile([128, W], f16, tag="lnS", name="lnS0")
        nc.scalar.activation(lnS_cur[0][:, :], Uta[:, :], AF.Ln, bias=s_pre[:, 0, 6:7])
        t_cur[0] = sbr.tile([128, W], f16, tag="tt", name="t0")
        nc.vector.tensor_tensor(t_cur[0][:, :], prod[0][:, :], lnS_cur[0][:, :], Op.subtract)

        import concourse.bass as bass_mod
        KL_AT = {5: 0, 9: 1, 13: 2, 15: 3}   # stage -> chunk emitted as fill

        # ---------- stages ----------
        for j in range(NT):
            if j + 2 < NT:
                work.extend(make_prod(j + 2))
            if j in KL_AT:
                work.extend(make_kl(KL_AT[j]))

            t_j = t_cur.pop(j)
            top8v = sbit.tile([128, 8], f16, tag="st_top8v")
            nc.vector.max(top8v[:, :], t_j[:, :])
            pos8 = sbit.tile([128, 8], mybir.dt.uint32, tag="st_pos8")
            nc.vector.max_index(pos8[:, :], top8v[:, :], t_j[:, :])

            tid = sbit.tile([128, 1], f32, tag="st_tid")
            nc.vector.tensor_copy(tid[:, :], pos8[:, 0:1])
            act = sbit.tile([128, 1], f32, tag="st_act")
            nc.vector.tensor_scalar(act[:, :], top8v[:, 0:1], float(LN_THR), None, Op.is_gt)
            te1 = sbit.tile([128, 1], f32, tag="st_te1")
            nc.vector.scalar_tensor_tensor(te1[:, :], tid[:, :], c_p1[:, 0:1], act[:, :], Op.add, Op.mult)
            tid_eff = sbit.tile([128, 1], f32, tag="st_tideff")
            nc.vector.tensor_scalar(tid_eff[:, :], te1[:, :], c_p1[:, 0:1], None, Op.subtract)
            # one-hot over W of this stage's proposals: every act=1 proposal's
            # teacher is accepted by its earliest proposer, so proposals and
            # accepts kill the same columns — Uta commits before the conflict
            # round trip resolves.
            # PE: proposal broadcast (transpose) + Uta kill commit + conflict matrix
            tp = ps.tile([128, 128], f32, tag="ps_tp", name="ittp")
            nc.tensor.transpose(tp[0:1, 0:128], tid_eff[:, 0:1], c_id[:, :])
            if j + 1 < NT:
                ohw = sbit.tile([128, W], f16, tag="st_ohw")
                nc.vector.tensor_scalar(ohw[:, :], tco[:, 4, :], tid_eff[:, 0:1], None, Op.is_equal)
                for (o, n) in CH:
                    nc.tensor.matmul(Uta[:, o:o+n], c_kbig[:, :], ohw[:, o:o+n],
                                     start=False, stop=True, skip_group_check=True)

            if j + 1 < NT:
                lnS_cur[j + 1] = sbb.tile([128, W], f16, tag="lnS", name=f"lnS{j+1}")
                nc.scalar.activation(lnS_cur[j + 1][:, :], Uta[:, :], AF.Ln, bias=s_pre[:, j+1, 6:7])

            itrow = sbit.tile([1, 128], f32, tag="st_itrow")
            nc.scalar.copy(itrow[:1, :], tp[0:1, 0:128])
            trep = ps.tile([128, 128], f32, tag="ps_trep", name="ittrep")
            nc.tensor.matmul(trep[:, :], c_ones1[:1, :], itrow[:1, :])

            if j + 1 < NT:
                # drain fill until lni_{j+1} is emitted, then chain t_{j+1}
                while work and (j + 1) not in prod:
                    work.popleft()()
                t_cur[j + 1] = sbr.tile([128, W], f16, tag="tt", name=f"t{j+1}")
                nc.vector.tensor_tensor(t_cur[j + 1][:, :], prod.pop(j + 1)[:, :],
                                        lnS_cur.pop(j + 1)[:, :], Op.subtract)

            # conflict: an earlier partition proposes the same teacher -> lost
            cnt = sbit.tile([128, 1], f32, tag="st_cnt")
            escr = sbit.tile([128, 128], f32, tag="st_escr")
            nc.vector.scalar_tensor_tensor(escr[:, :], trep[:, :], tid_eff[:, 0:1], c_lt[:, :],
                                           Op.is_equal, Op.mult, accum_out=cnt[:, 0:1])
            notlost = sbit.tile([128, 1], f32, tag="st_nl")
            nc.vector.tensor_scalar(notlost[:, :], cnt[:, :], 0.5, None, Op.is_le)
            nc.vector.tensor_tensor(w_all[:, j:j+1], act[:, :], notlost[:, :], Op.mult)
            tsp1 = sbit.tile([128, 1], f32, tag="st_tsp1")
            nc.vector.scalar_tensor_tensor(tsp1[:, :], tid[:, :], 1.0, w_all[:, j:j+1], Op.add, Op.mult)
            tid_sel = sbit.tile([128, 1], f32, tag="st_tidsel")
            nc.vector.tensor_scalar(tid_sel[:, :], tsp1[:, :], 1.0, None, Op.subtract)

            # gather matched teacher rows from DRAM by index (idle DMA engines)
            tidc = sbit.tile([128, 1], f32, tag="st_tidc")
            nc.gpsimd.tensor_scalar(tidc[:, :], tid_sel[:, :], 0.0, None, Op.max)
            tidi = sbit.tile([128, 1], mybir.dt.int32, tag="st_tidi")
            nc.gpsimd.tensor_copy(tidi[:, :], tidc[:, :])
            gc, gi = g_of(j)
            nc.gpsimd.indirect_dma_start(
                out=Gs[gc][:, gi, :], out_offset=None,
                in_=t_rows_nat.ap()[:, :],
                in_offset=bass_mod.IndirectOffsetOnAxis(ap=tidi[:, 0:1], axis=0),
            )

            if j + 1 < NT:
                pump(3)  # keep KL-chunk backlog from piling onto the chain
            else:
                pump_all()

        pump_all()

        # ---------- epilogue ----------
        # last KL chunk (tile 15 only)
        for fn in make_kl(4):
            fn()

        # klD = ln se - ln tse
        lnse = sbe.tile([128, NT], f32)
        nc.scalar.activation(lnse[:, :], se_all[:, :], AF.Ln)
        lntse = sbe.tile([128, NT], f32)
        nc.scalar.activation(lntse[:, :], tse_all[:, :], AF.Ln)
        klD = sbe.tile([128, NT], f32)
        nc.vector.tensor_tensor(klD[:, :], lnse[:, :], lntse[:, :], Op.subtract)

        # kl = 0.25*(klA - klB)/tse + klD, weighted by w
        kl = sbe.tile([128, NT], f32)
        nc.vector.tensor_tensor(kl[:, :], klA[:, :], klB[:, :], Op.subtract)
        rtse = sbe.tile([128, NT], f32)
        nc.vector.reciprocal(rtse[:, :], tse_all[:, :])
        nc.vector.tensor_scalar(rtse[:, :], rtse[:, :], 1.0 / TEMP, None, Op.mult)
        nc.vector.tensor_tensor(kl[:, :], kl[:, :], rtse[:, :], Op.mult)
        nc.vector.tensor_tensor(kl[:, :], kl[:, :], klD[:, :], Op.add)
        nc.vector.tensor_tensor(kl[:, :], kl[:, :], w_all[:, :], Op.mult)

        # --- exact miou recompute + box/conf (chunked over the 4 G tiles) ---
        gx1 = sbe.tile([128, NT], f32); gx2 = sbe.tile([128, NT], f32)
        gy1 = sbe.tile([128, NT], f32); gy2 = sbe.tile([128, NT], f32)
        gta = sbe.tile([128, NT], f32)
        e1 = sbe.tile([128, NT], f32); e2 = sbe.tile([128, NT], f32)
        for c, (o, nn) in enumerate(GS_SPEC):
            T = slice(o, o + nn)
            nc.vector.scalar_tensor_tensor(gx1[:, T], Gs[c][:, :, 2], -0.5, Gs[c][:, :, 0], Op.mult, Op.add)
            nc.vector.scalar_tensor_tensor(gx2[:, T], Gs[c][:, :, 2], 0.5, Gs[c][:, :, 0], Op.mult, Op.add)
            nc.vector.scalar_tensor_tensor(gy1[:, T], Gs[c][:, :, 3], -0.5, Gs[c][:, :, 1], Op.mult, Op.add)
            nc.vector.scalar_tensor_tensor(gy2[:, T], Gs[c][:, :, 3], 0.5, Gs[c][:, :, 1], Op.mult, Op.add)
        nc.vector.tensor_tensor(e1[:, :], gx2[:, :], gx1[:, :], Op.subtract)
        nc.vector.tensor_tensor(e2[:, :], gy2[:, :], gy1[:, :], Op.subtract)
        nc.vector.tensor_tensor(gta[:, :], e1[:, :], e2[:, :], Op.mult)
        m1 = sbe.tile([128, NT], f32); m2 = sbe.tile([128, NT], f32)
        whx = sbe.tile([128, NT], f32); why = sbe.tile([128, NT], f32)
        nc.vector.tensor_tensor(m1[:, :], gx1[:, :], s_pre[:, :, 0], Op.max)
        nc.vector.tensor_tensor(m2[:, :], gx2[:, :], s_pre[:, :, 1], Op.min)
        nc.vector.tensor_tensor(whx[:, :], m2[:, :], m1[:, :], Op.subtract)
        nc.scalar.activation(whx[:, :], whx[:, :], AF.Relu)
        nc.vector.tensor_tensor(m1[:, :], gy1[:, :], s_pre[:, :, 2], Op.max)
        nc.vector.tensor_tensor(m2[:, :], gy2[:, :], s_pre[:, :, 3], Op.min)
        nc.vector.tensor_tensor(why[:, :], m2[:, :], m1[:, :], Op.subtract)
        nc.scalar.activation(why[:, :], why[:, :], AF.Relu)
        inter = sbe.tile([128, NT], f32)
        nc.vector.tensor_tensor(inter[:, :], whx[:, :], why[:, :], Op.mult)
        den = sbe.tile([128, NT], f32)
        nc.vector.tensor_tensor(den[:, :], s_pre[:, :, 5], gta[:, :], Op.add)
        nc.vector.scalar_tensor_tensor(den[:, :], den[:, :], 1e-7, inter[:, :], Op.add, Op.subtract)
        nc.vector.reciprocal(den[:, :], den[:, :])
        miou = sbe.tile([128, NT], f32)
        nc.vector.tensor_tensor(miou[:, :], inter[:, :], den[:, :], Op.mult)
        nc.vector.tensor_tensor(miou[:, :], miou[:, :], w_all[:, :], Op.mult)

        # box loss: sum |s-t| over 4 coords * miou * w
        bsum = sbe.tile([128, NT], f32)
        bd = sbe.tile([128, NT], f32)
        for col in range(4):
            for c, (o, nn) in enumerate(GS_SPEC):
                T = slice(o, o + nn)
                nc.vector.tensor_tensor(bd[:, T], s_c[:, T, col], Gs[c][:, :, col], Op.subtract)
            nc.scalar.activation(bd[:, :], bd[:, :], AF.Abs)
            if col == 0:
                nc.vector.tensor_copy(bsum[:, :], bd[:, :])
            else:
                nc.vector.tensor_tensor(bsum[:, :], bsum[:, :], bd[:, :], Op.add)
        nc.vector.tensor_tensor(bsum[:, :], bsum[:, :], miou[:, :], Op.mult)

        # conf loss: (s_conf - t_conf*miou)^2 * w   (miou already w-masked)
        cf = sbe.tile([128, NT], f32)
        for c, (o, nn) in enumerate(GS_SPEC):
            T = slice(o, o + nn)
            nc.vector.tensor_tensor(cf[:, T], Gs[c][:, :, 4], miou[:, T], Op.mult)
        nc.vector.tensor_tensor(cf[:, :], s_c[:, :, 4], cf[:, :], Op.subtract)
        nc.vector.tensor_tensor(cf[:, :], cf[:, :], cf[:, :], Op.mult)
        nc.vector.tensor_tensor(cf[:, :], cf[:, :], w_all[:, :], Op.mult)

        # reductions
        acc = sbe.tile([128, 4], f32)
        nc.vector.reduce_sum(acc[:, 0:1], kl[:, :], axis=AX.X)
        nc.vector.reduce_sum(acc[:, 1:2], bsum[:, :], axis=AX.X)
        nc.vector.reduce_sum(acc[:, 2:3], cf[:, :], axis=AX.X)
        nc.vector.reduce_sum(acc[:, 3:4], w_all[:, :], axis=AX.X)
        accp_full = ps.tile([1, 512], f32, tag="ps_acc", name="accp")
        accrow = accp_full[0:1, 0:4]
        nc.tensor.matmul(accrow[0:1, :], c_ones_col[:, 0:1], acc[:, :])
        res = sbe.tile([1, 8], f32)
        nc.vector.memset(res[:1, :], 0.0)
        nc.vector.tensor_copy(res[:1, 0:4], accrow[0:1, 0:4])
        Msafe = sbe.tile([1, 1], f32, tag="msafe")
        nc.vector.tensor_scalar(Msafe[:1, :], res[:1, 3:4], 1.0, None, Op.max)
        nc.vector.reciprocal(Msafe[:1, :], Msafe[:1, :])
        nc.vector.tensor_scalar(res[:1, 4:5], Msafe[:1, :], 1.0, None, Op.mult)
        nc.sync.dma_start(out.ap()[:, :], res[:1, :])

    nc.compile()
    return nc


def _consts():
    f32 = np.float32
    if "consts" not in _CACHE:
        _CACHE["consts"] = {
            "p1col": (np.arange(128, dtype=f32)[:, None] + 1.0),
            "ltmask": np.tril(np.ones((128, 128), f32), -1),
            "identity": np.eye(128, dtype=f32),
            "ones_col": np.ones((1, 128), f32),
            "ones128_col": np.ones((128, 1), f32),
            "kbig_lhs": np.full((128, 128), KILLV, np.float16),
        }
    return _CACHE["consts"]


def _prep_core_inputs(s_img, t_img):
    """Build per-core inputs from one (padded) student image [2048, 85] and
    the ORIGINAL teacher rows (1024 or 2048, uncompacted)."""
    f32 = np.float32
    W = _CACHE["W"]
    s = np.asarray(s_img, f32)
    t = np.asarray(t_img, f32)

    # teacher compaction (order-preserving), reference conf>0.5 + fallback
    mask = t[:, 4] > 0.5
    if not mask.any():
        mask = np.zeros_like(mask)
        mask[int(np.argmax(t[:, 4]))] = True
    vidx = np.where(mask)[0]
    nv = len(vidx)
    assert nv <= W, f"valid teachers {nv} > W={W}"
    tc = t[vidx]

    tx1 = tc[:, 0] - tc[:, 2] / f32(2); tx2 = tc[:, 0] + tc[:, 2] / f32(2)
    ty1 = tc[:, 1] - tc[:, 3] / f32(2); ty2 = tc[:, 1] + tc[:, 3] / f32(2)
    ta = ((tx2 - tx1) * (ty2 - ty1)).astype(f32)

    tcoords = np.zeros((5, W), np.float16)
    tcoords[0, :nv] = tx1; tcoords[1, :nv] = tx2
    tcoords[2, :nv] = ty1; tcoords[3, :nv] = ty2
    tcoords[4, :] = np.arange(W, dtype=f32)
    tcoords_rep = np.broadcast_to(tcoords[None, :, :], (128, 5, W)).copy()

    ta_row = np.full((1, W), 4.0, f32)
    ta_row[0, :nv] = ta
    inv_row = np.zeros((1, W), np.float16)
    inv_row[0, nv:] = 1.0

    t_rows_nat = np.zeros((W, D), f32)
    t_rows_nat[:nv] = tc

    s_cols = np.empty((128, NT, 5), f32)
    s_logits = np.empty((128, NT, 80), f32)
    for j in range(NT):
        s_cols[:, j, :] = s[j*128:(j+1)*128, :5]
        s_logits[:, j, :] = s[j*128:(j+1)*128, 5:]

    sx1 = s_cols[:, :, 0] - s_cols[:, :, 2] / 2
    sx2 = s_cols[:, :, 0] + s_cols[:, :, 2] / 2
    sy1 = s_cols[:, :, 1] - s_cols[:, :, 3] / 2
    sy2 = s_cols[:, :, 1] + s_cols[:, :, 3] / 2
    sa = ((sx2 - sx1) * (sy2 - sy1)).astype(f32)
    s_pre = np.stack([sx1, sx2, sy1, sy2, -sx1, sa, sa + f32(1e-7)], axis=-1).astype(f32)

    return {
        "tcoords": tcoords_rep, "ta_row": ta_row, "inv_row": inv_row,
        "s_cols": s_cols, "s_pre": s_pre, "s_logits": s_logits,
        "t_rows_nat": t_rows_nat, **_consts(),
    }, vidx


def _pad_scale1(s):
    """Pad students [1024, 85] -> [2048, 85] with inert rows."""
    f32 = np.float32
    ns = np.zeros((NS, D), f32)
    ns[:s.shape[0]] = s
    ns[s.shape[0]:, 0] = PAD_X
    ns[s.shape[0]:, 2] = 1.0
    ns[s.shape[0]:, 3] = 1.0
    return ns


def _max_nv(*teachers):
    best = 1
    for t in teachers:
        for b in range(t.shape[0]):
            best = max(best, int((t[b, :, 4] > 0.5).sum()))
    return best


def kernel(student_out0, teacher_out0, student_out1, teacher_out1):
    from concourse.bass_utils import run_bass_kernel_spmd

    student_out0 = np.asarray(student_out0, np.float32)
    teacher_out0 = np.asarray(teacher_out0, np.float32)
    student_out1 = np.asarray(student_out1, np.float32)
    teacher_out1 = np.asarray(teacher_out1, np.float32)

    W = (_max_nv(teacher_out0, teacher_out1) + 63) // 64 * 64
    if _CACHE.get("W") != W:
        _CACHE["W"] = W
        _CACHE["nc"] = _build_nc(W)
    nc = _CACHE["nc"]

    in_maps = []
    for c in range(4):
        m, _ = _prep_core_inputs(student_out0[c], teacher_out0[c])
        in_maps.append(m)
    for c in range(4):
        m, _ = _prep_core_inputs(_pad_scale1(student_out1[c]), teacher_out1[c])
        in_maps.append(m)

    res = run_bass_kernel_spmd(nc, in_maps, core_ids=list(range(8)))

    cls_t = box_t = conf_t = nm = np.float32(0.0)
    for c in range(8):
        o = res.results[c]["out"][0]
        kl_s, box_s, conf_s, M, minv = o[0], o[1], o[2], o[3], o[4]
        cls_t += np.float32(kl_s) * np.float32(minv) * np.float32(TEMP * TEMP)
        box_t += np.float32(box_s) * np.float32(minv) / np.float32(4.0)
        conf_t += np.float32(conf_s) * np.float32(minv)
        nm += np.float32(M)
    nms = max(nm, np.float32(1.0))
    cls_t, box_t, conf_t = cls_t / nms, box_t / nms, conf_t / nms
    total = np.float32(ALPHA) * cls_t + np.float32(BETA) * box_t + np.float32(1.0 - ALPHA - BETA) * conf_t
    return np.float32(total)


# revision 14
# speedup vs baseline: 1.2070x; 1.0262x over previous
"""CrossKD loss kernel for Trainium2, 8 NeuronCores — v4.

Sharding: one (image, scale) pair per core. Cores 0-3: scale-0 images
(2048 anchors); cores 4-7: scale-1 images (1024 anchors) padded to 2048
students with inert rows. One SPMD program on all 8 cores.

v4 changes vs v3 (239us):
  * Used-mask folded into the Ln denominator: a PSUM tensor
    Uta = ta + 65504*(#kills) is maintained by the PE kill-matmuls, and
    lnS = Ln(Uta + sa) de-ranks killed columns (ln(65504) ~ 11 pushes
    them far below the -1.1 match threshold, and they can never pass it
    since lni <= ~0.8). This deletes both the per-stage U16 scalar copy
    and the full-width av add on vector.
  * W dynamic: ceil64(max valid teachers) instead of hardcoded 1152.
  * Teacher coord rows (tx1,tx2,ty1,ty2,iota) shipped pre-replicated
    [128,5,W] f16 over DMA — the startup PE/scalar replicate cascade is
    gone; only ta + invalid-mask enter via K=1 matmuls (Uta init).
  * x-axis intersection via Scalar relu-form: whx = relu(sw - a - b),
    a = relu(tx1-sx1), b = relu(sx2-tx2) — two TS + one TT leave the
    (bottleneck) vector engine for the (slack) scalar engine.
  * KL loss computed in 4-tile chunks pumped as fill work inside the
    stage loop (chunks 0-2); only the last chunk runs in the epilogue.
Host: sums the 4 accumulators over 8 cores, normalizes, weighted sum.
"""
import numpy as np

ALPHA, BETA, TEMP = 0.6, 0.3, 4.0
LN_THR = -1.0986122886681098   # iou > 0.5  <=>  r > 1/3  <=>  ln r > ln(1/3)
KILLV = 65504.0                # f16 max; ln(ta+KILLV+sa) ~ 11.1 >> |LN_THR|
NS = 2048                # padded students per core
NT = 16                  # student tiles
D = 85

PAD_X = 30000.0          # inert-student x center (fp16-safe)

_CACHE = {}


def _build_nc(W):
    import concourse.bacc as bacc
    import concourse.mybir as mybir
    from concourse.tile import TileContext
    from concourse.alu_op_type import AluOpType as Op
    dt = mybir.dt
    AF = mybir.ActivationFunctionType
    AX = mybir.AxisListType
    f32 = dt.float32
    f16 = dt.float16

    nc = bacc.Bacc("TRN2", num_devices=8, debug=False)

    # ---- DRAM I/O ----
    tcoords = nc.dram_tensor("tcoords", [128, 5, W], f16, kind="ExternalInput")  # tx1,tx2,ty1,ty2,iota
    ta_row_d = nc.dram_tensor("ta_row", [1, W], f32, kind="ExternalInput")
    inv_row_d = nc.dram_tensor("inv_row", [1, W], f16, kind="ExternalInput")
    s_cols = nc.dram_tensor("s_cols", [128, NT, 5], f32, kind="ExternalInput")
    s_pre_d = nc.dram_tensor("s_pre", [128, NT, 7], f32, kind="ExternalInput")  # sx1,sx2,sy1,sy2,nsx1,sa,sa1e7
    s_logits = nc.dram_tensor("s_logits", [128, NT, 80], f32, kind="ExternalInput")
    t_rows_nat = nc.dram_tensor("t_rows_nat", [W, D], f32, kind="ExternalInput")
    p1col = nc.dram_tensor("p1col", [128, 1], f32, kind="ExternalInput")      # p+1
    ltmask = nc.dram_tensor("ltmask", [128, 128], f32, kind="ExternalInput")  # strict lower tri
    identity = nc.dram_tensor("identity", [128, 128], f32, kind="ExternalInput")
    ones_col = nc.dram_tensor("ones_col", [1, 128], f32, kind="ExternalInput")
    ones128_col = nc.dram_tensor("ones128_col", [128, 1], f32, kind="ExternalInput")
    kbig_lhs = nc.dram_tensor("kbig_lhs", [128, 128], f16, kind="ExternalInput")  # 65504
    out = nc.dram_tensor("out", [1, 8], f32, kind="ExternalOutput")

    # PSUM-bank-aligned accumulation chunks
    CH = []
    o = 0
    while o < W:
        n = min(512, W - o)
        CH.append((o, n))
        o += n

    from contextlib import ExitStack
    with TileContext(nc) as tc, ExitStack() as stack:
        # Load the one activation table that serves every func used here
        # (exp, ln, relu, copy, abs) so the table-load pass never swaps.
        from concourse.hw_specs import get_activation_tables
        _tabs = list(get_activation_tables(nc.m.arch))
        nc.scalar.add_instruction(mybir.InstLoadActFuncSet(
            name=nc.scalar.bass.get_next_instruction_name(),
            act_func_set_id=_tabs.index("natural_log_exp_and_others")))
        sb = stack.enter_context(tc.tile_pool(name="sbp", bufs=1))
        ps = stack.enter_context(tc.tile_pool(name="ps", bufs=1, space="PSUM"))
        sbb = stack.enter_context(tc.tile_pool(name="sbb", bufs=3))
        sbr = stack.enter_context(tc.tile_pool(name="sbr", bufs=4))
        sbit = stack.enter_context(tc.tile_pool(name="sbit", bufs=2))
        sbe = stack.enter_context(tc.tile_pool(name="sbe", bufs=1))

        # ---------- inputs (critical-path first; big tco on the scalar queue) ----------
        tco = sb.tile([128, 5, W], f16)
        nc.scalar.dma_start(tco[:, :, :], tcoords.ap()[:, :, :])
        s_pre = sb.tile([128, NT, 7], f32)
        nc.sync.dma_start(s_pre[:, :, :], s_pre_d.ap()[:, :, :])
        s_c = sb.tile([128, NT, 5], f32)
        nc.sync.dma_start(s_c[:, :, :], s_cols.ap()[:, :, :])
        c_ones1 = sb.tile([1, 128], f32); nc.sync.dma_start(c_ones1[:, :], ones_col.ap()[:, :])
        ta_row = sb.tile([1, W], f32)
        nc.sync.dma_start(ta_row[:1, :], ta_row_d.ap()[:, :])
        inv_row = sb.tile([1, W], f16)
        nc.sync.dma_start(inv_row[:1, :], inv_row_d.ap()[:, :])
        c_kbig = sb.tile([128, 128], f16); nc.sync.dma_start(c_kbig[:, :], kbig_lhs.ap()[:, :])
        c_p1 = sb.tile([128, 1], f32); nc.sync.dma_start(c_p1[:, :], p1col.ap()[:, :])
        c_lt = sb.tile([128, 128], f32); nc.sync.dma_start(c_lt[:, :], ltmask.ap()[:, :])
        c_id = sb.tile([128, 128], f32); nc.sync.dma_start(c_id[:, :], identity.ap()[:, :])
        c_ones_col = sb.tile([128, 1], f32); nc.sync.dma_start(c_ones_col[:, :], ones128_col.ap()[:, :])
        slg = sb.tile([128, NT, 80], f32)
        nc.sync.dma_start(slg[:, :, :], s_logits.ap()[:, :, :])

        # ---------- Uta init: PSUM = ta + KILLV*invalid ----------
        Uta = ps.tile([128, W], f32, tag="ps_U", name="Uta")
        for (o, n) in CH:
            nc.tensor.matmul(Uta[:, o:o+n], c_ones1[:1, :], ta_row[0:1, o:o+n],
                             start=True, stop=True, skip_group_check=True)
        for (o, n) in CH:
            nc.tensor.matmul(Uta[:, o:o+n], c_kbig[0:1, :], inv_row[:1, o:o+n],
                             start=False, stop=True, skip_group_check=True)

        # student scalars are host-precomputed planes of s_pre:
        # 0:sx1 1:sx2 2:sy1 3:sy2 4:nsx1 5:sa 6:sa1e7

        # ---------- per-stage results ----------
        w_all = sb.tile([128, NT], f32)
        GS_SPEC = [(0, 4), (4, 4), (8, 4), (12, 3), (15, 1)]
        Gs = [sb.tile([128, n, D], f32, tag=f"G{c}", name=f"G{c}")
              for c, (o, n) in enumerate(GS_SPEC)]

        def g_of(j):
            for c, (o, n) in enumerate(GS_SPEC):
                if o <= j < o + n:
                    return c, j - o
            raise AssertionError(j)
        se_all = sb.tile([128, NT], f32)
        tse_all = sb.tile([128, NT], f32)
        klA = sb.tile([128, NT], f32)
        klB = sb.tile([128, NT], f32)

        # ---------- production of lni tiles (fill work) ----------
        prod = {}

        def make_prod(j):
            st = {}

            def p_a():
                st["a"] = sbb.tile([128, W], f16, tag="pa", name=f"a{j}")
                nc.scalar.activation(st["a"][:, :], tco[:, 0, :], AF.Relu,
                                     bias=s_pre[:, j, 4:5])

            def p_b():
                st["b"] = sbb.tile([128, W], f16, tag="pb", name=f"b{j}")
                nc.scalar.activation(st["b"][:, :], tco[:, 1, :], AF.Relu,
                                     scale=-1.0, bias=s_pre[:, j, 1:2])

            def p_cx():
                st["cx"] = sbb.tile([128, W], f16, tag="pcx", name=f"cx{j}")
                nc.vector.tensor_tensor(st["cx"][:, :], st["a"][:, :], st["b"][:, :], Op.add)

            def p_whx():
                st["whx"] = sbb.tile([128, W], f16, tag="pwhx", name=f"whx{j}")
                nc.scalar.activation(st["whx"][:, :], st["cx"][:, :], AF.Relu,
                                     scale=-1.0, bias=s_c[:, j, 2:3])

            def p_m1y():
                st["m1y"] = sbb.tile([128, W], f16, tag="pm1y", name=f"m1y{j}")
                nc.vector.tensor_scalar(st["m1y"][:, :], tco[:, 2, :], s_pre[:, j, 2:3], None, Op.max)

            def p_t1y():
                st["t1y"] = sbb.tile([128, W], f16, tag="pt1y", name=f"t1y{j}")
                nc.vector.tensor_scalar(st["t1y"][:, :], tco[:, 3, :], s_pre[:, j, 3:4], None, Op.min)

            def p_wyr():
                st["wyr"] = sbb.tile([128, W], f16, tag="pwyr", name=f"wyr{j}")
                nc.vector.tensor_tensor(st["wyr"][:, :], st["t1y"][:, :], st["m1y"][:, :], Op.subtract)

            def p_why():
                st["why"] = sbb.tile([128, W], f16, tag="pwhy", name=f"why{j}")
                nc.scalar.activation(st["why"][:, :], st["wyr"][:, :], AF.Relu)

            def p_inter():
                st["inter"] = sbb.tile([128, W], f16, tag="pinter", name=f"inter{j}")
                nc.vector.tensor_tensor(st["inter"][:, :], st["whx"][:, :], st["why"][:, :], Op.mult)

            def p_lni():
                prod[j] = sbr.tile([128, W], f16, tag="lni", name=f"lni{j}")
                nc.scalar.activation(prod[j][:, :], st["inter"][:, :], AF.Ln)

            return [p_a, p_b, p_m1y, p_t1y, p_cx, p_wyr, p_whx, p_why, p_inter, p_lni]

        # ---------- KL chunk closures (fill work; chunk c = tiles 4c..4c+3) ----------
        def make_kl(c):
            o, nn = GS_SPEC[c]
            T = slice(o, o + nn)
            st = {}

            def k_sexp():
                st["sex"] = sbb.tile([128, 4, 80], f32, tag="ksex", name=f"sex{c}")
                nc.scalar.activation(st["sex"][:, 0:nn, :], slg[:, T, :], AF.Exp, scale=1.0 / TEMP)

            def k_texp():
                st["tex"] = sbb.tile([128, 4, 80], f32, tag="ktex", name=f"tex{c}")
                nc.scalar.activation(st["tex"][:, 0:nn, :], Gs[c][:, :, 5:], AF.Exp, scale=1.0 / TEMP)

            def k_se():
                nc.vector.tensor_reduce(se_all[:, T], st["sex"][:, 0:nn, :], AX.X, Op.add)

            def k_tse():
                nc.vector.tensor_reduce(tse_all[:, T], st["tex"][:, 0:nn, :], AX.X, Op.add)

            def k_pa():
                st["pA"] = sbb.tile([128, 4, 80], f32, tag="kpA", name=f"pA{c}")
                nc.vector.tensor_tensor(st["pA"][:, 0:nn, :], st["tex"][:, 0:nn, :], Gs[c][:, :, 5:], Op.mult)

            def k_ka():
                nc.vector.tensor_reduce(klA[:, T], st["pA"][:, 0:nn, :], AX.X, Op.add)

            def k_pb():
                st["pB"] = sbb.tile([128, 4, 80], f32, tag="kpB", name=f"pB{c}")
                nc.vector.tensor_tensor(st["pB"][:, 0:nn, :], st["tex"][:, 0:nn, :], slg[:, T, :], Op.mult)

            def k_kb():
                nc.vector.tensor_reduce(klB[:, T], st["pB"][:, 0:nn, :], AX.X, Op.add)

            return [k_sexp, k_texp, k_se, k_tse, k_pa, k_ka, k_pb, k_kb]

        # ---------- work queue ----------
        from collections import deque
        work = deque()

        def pump(n):
            for _ in range(n):
                if work:
                    work.popleft()()
                else:
                    return

        def pump_all():
            while work:
                work.popleft()()

        # prime: production 0 fully, productions 1-2 queued
        for fn in make_prod(0):
            fn()
        work.extend(make_prod(1))
        work.extend(make_prod(2))
        pump(4)

        # lnS_0 / t_0
        lnS_cur = {}
        t_cur = {}
        lnS_cur[0] = sbb.tile([128, W], f16, tag="lnS", name="lnS0")
        nc.scalar.activation(lnS_cur[0][:, :], Uta[:, :], AF.Ln, bias=s_pre[:, 0, 6:7])
        t_cur[0] = sbr.tile([128, W], f16, tag="tt", name="t0")
        nc.vector.tensor_tensor(t_cur[0][:, :], prod[0][:, :], lnS_cur[0][:, :], Op.subtract)

        import concourse.bass as bass_mod
        KL_AT = {5: 0, 9: 1, 13: 2, 15: 3}   # stage -> chunk emitted as fill

        # ---------- stages ----------
        for j in range(NT):
            if j + 3 < NT:
                work.extend(make_prod(j + 3))
            if j in KL_AT:
                work.extend(make_kl(KL_AT[j]))

            t_j = t_cur.pop(j)
            top8v = sbit.tile([128, 8], f16, tag="st_top8v")
            nc.vector.max(top8v[:, :], t_j[:, :])
            pos8 = sbit.tile([128, 8], mybir.dt.uint32, tag="st_pos8")
            nc.vector.max_index(pos8[:, :], top8v[:, :], t_j[:, :])

            tid = sbit.tile([128, 1], f32, tag="st_tid")
            nc.vector.tensor_copy(tid[:, :], pos8[:, 0:1])
            act = sbit.tile([128, 1], f32, tag="st_act")
            nc.vector.tensor_scalar(act[:, :], top8v[:, 0:1], float(LN_THR), None, Op.is_gt)
            te1 = sbit.tile([128, 1], f32, tag="st_te1")
            nc.vector.scalar_tensor_tensor(te1[:, :], tid[:, :], c_p1[:, 0:1], act[:, :], Op.add, Op.mult)
            tid_eff = sbit.tile([128, 1], f32, tag="st_tideff")
            nc.vector.tensor_scalar(tid_eff[:, :], te1[:, :], c_p1[:, 0:1], None, Op.subtract)
            # one-hot over W of this stage's proposals: every act=1 proposal's
            # teacher is accepted by its earliest proposer, so proposals and
            # accepts kill the same columns — Uta commits before the conflict
            # round trip resolves.
            # PE: proposal broadcast (transpose) + Uta kill commit + conflict matrix
            tp = ps.tile([128, 128], f32, tag="ps_tp", name="ittp")
            nc.tensor.transpose(tp[0:1, 0:128], tid_eff[:, 0:1], c_id[:, :])
            if j + 1 < NT:
                ohw = sbit.tile([128, W], f16, tag="st_ohw")
                nc.vector.tensor_scalar(ohw[:, :], tco[:, 4, :], tid_eff[:, 0:1], None, Op.is_equal)
                for (o, n) in CH:
                    nc.tensor.matmul(Uta[:, o:o+n], c_kbig[:, :], ohw[:, o:o+n],
                                     start=False, stop=True, skip_group_check=True)

            if j + 1 < NT:
                lnS_cur[j + 1] = sbb.tile([128, W], f16, tag="lnS", name=f"lnS{j+1}")
                nc.scalar.activation(lnS_cur[j + 1][:, :], Uta[:, :], AF.Ln, bias=s_pre[:, j+1, 6:7])

            itrow = sbit.tile([1, 128], f32, tag="st_itrow")
            nc.scalar.copy(itrow[:1, :], tp[0:1, 0:128])
            trep = ps.tile([128, 128], f32, tag="ps_trep", name="ittrep")
            nc.tensor.matmul(trep[:, :], c_ones1[:1, :], itrow[:1, :])

            if j + 1 < NT:
                # drain fill until lni_{j+1} is emitted, then chain t_{j+1}
                while work and (j + 1) not in prod:
                    work.popleft()()
                t_cur[j + 1] = sbr.tile([128, W], f16, tag="tt", name=f"t{j+1}")
                nc.vector.tensor_tensor(t_cur[j + 1][:, :], prod.pop(j + 1)[:, :],
                                        lnS_cur.pop(j + 1)[:, :], Op.subtract)

            # conflict: an earlier partition proposes the same teacher -> lost
            cnt = sbit.tile([128, 1], f32, tag="st_cnt")
            escr = sbit.tile([128, 128], f32, tag="st_escr")
            nc.vector.scalar_tensor_tensor(escr[:, :], trep[:, :], tid_eff[:, 0:1], c_lt[:, :],
                                           Op.is_equal, Op.mult, accum_out=cnt[:, 0:1])
            notlost = sbit.tile([128, 1], f32, tag="st_nl")
            nc.vector.tensor_scalar(notlost[:, :], cnt[:, :], 0.5, None, Op.is_le)
            nc.vector.tensor_tensor(w_all[:, j:j+1], act[:, :], notlost[:, :], Op.mult)
            tsp1 = sbit.tile([128, 1], f32, tag="st_tsp1")
            nc.vector.scalar_tensor_tensor(tsp1[:, :], tid[:, :], 1.0, w_all[:, j:j+1], Op.add, Op.mult)
            tid_sel = sbit.tile([128, 1], f32, tag="st_tidsel")
            nc.vector.tensor_scalar(tid_sel[:, :], tsp1[:, :], 1.0, None, Op.subtract)

            # gather matched teacher rows from DRAM by index (idle DMA engines)
            tidc = sbit.tile([128, 1], f32, tag="st_tidc")
            nc.gpsimd.tensor_scalar(tidc[:, :], tid_sel[:, :], 0.0, None, Op.max)
            tidi = sbit.tile([128, 1], mybir.dt.int32, tag="st_tidi")
            nc.gpsimd.tensor_copy(tidi[:, :], tidc[:, :])
            gc, gi = g_of(j)
            nc.gpsimd.indirect_dma_start(
                out=Gs[gc][:, gi, :], out_offset=None,
                in_=t_rows_nat.ap()[:, :],
                in_offset=bass_mod.IndirectOffsetOnAxis(ap=tidi[:, 0:1], axis=0),
            )

            if j + 1 < NT:
                pump(3)  # keep KL-chunk backlog from piling onto the chain
            else:
                pump_all()

        pump_all()

        # ---------- epilogue ----------
        # last KL chunk (tile 15 only)
        for fn in make_kl(4):
            fn()

        # klD = ln se - ln tse
        lnse = sbe.tile([128, NT], f32)
        nc.scalar.activation(lnse[:, :], se_all[:, :], AF.Ln)
        lntse = sbe.tile([128, NT], f32)
        nc.scalar.activation(lntse[:, :], tse_all[:, :], AF.Ln)
        klD = sbe.tile([128, NT], f32)
        nc.vector.tensor_tensor(klD[:, :], lnse[:, :], lntse[:, :], Op.subtract)

        # kl = 0.25*(klA - klB)/tse + klD, weighted by w
        kl = sbe.tile([128, NT], f32)
        nc.vector.tensor_tensor(kl[:, :], klA[:, :], klB[:, :], Op.subtract)
        rtse = sbe.tile([128, NT], f32)
        nc.vector.reciprocal(rtse[:, :], tse_all[:, :])
        nc.vector.tensor_scalar(rtse[:, :], rtse[:, :], 1.0 / TEMP, None, Op.mult)
        nc.vector.tensor_tensor(kl[:, :], kl[:, :], rtse[:, :], Op.mult)
        nc.vector.tensor_tensor(kl[:, :], kl[:, :], klD[:, :], Op.add)
        nc.vector.tensor_tensor(kl[:, :], kl[:, :], w_all[:, :], Op.mult)

        # --- exact miou recompute + box/conf (chunked over the 4 G tiles) ---
        gx1 = sbe.tile([128, NT], f32); gx2 = sbe.tile([128, NT], f32)
        gy1 = sbe.tile([128, NT], f32); gy2 = sbe.tile([128, NT], f32)
        gta = sbe.tile([128, NT], f32)
        e1 = sbe.tile([128, NT], f32); e2 = sbe.tile([128, NT], f32)
        for c, (o, nn) in enumerate(GS_SPEC):
            T = slice(o, o + nn)
            nc.vector.scalar_tensor_tensor(gx1[:, T], Gs[c][:, :, 2], -0.5, Gs[c][:, :, 0], Op.mult, Op.add)
            nc.vector.scalar_tensor_tensor(gx2[:, T], Gs[c][:, :, 2], 0.5, Gs[c][:, :, 0], Op.mult, Op.add)
            nc.vector.scalar_tensor_tensor(gy1[:, T], Gs[c][:, :, 3], -0.5, Gs[c][:, :, 1], Op.mult, Op.add)
            nc.vector.scalar_tensor_tensor(gy2[:, T], Gs[c][:, :, 3], 0.5, Gs[c][:, :, 1], Op.mult, Op.add)
        nc.vector.tensor_tensor(e1[:, :], gx2[:, :], gx1[:, :], Op.subtract)
        nc.vector.tensor_tensor(e2[:, :], gy2[:, :], gy1[:, :], Op.subtract)
        nc.vector.tensor_tensor(gta[:, :], e1[:, :], e2[:, :], Op.mult)
        m1 = sbe.tile([128, NT], f32); m2 = sbe.tile([128, NT], f32)
        whx = sbe.tile([128, NT], f32); why = sbe.tile([128, NT], f32)
        nc.vector.tensor_tensor(m1[:, :], gx1[:, :], s_pre[:, :, 0], Op.max)
        nc.vector.tensor_tensor(m2[:, :], gx2[:, :], s_pre[:, :, 1], Op.min)
        nc.vector.tensor_tensor(whx[:, :], m2[:, :], m1[:, :], Op.subtract)
        nc.scalar.activation(whx[:, :], whx[:, :], AF.Relu)
        nc.vector.tensor_tensor(m1[:, :], gy1[:, :], s_pre[:, :, 2], Op.max)
        nc.vector.tensor_tensor(m2[:, :], gy2[:, :], s_pre[:, :, 3], Op.min)
        nc.vector.tensor_tensor(why[:, :], m2[:, :], m1[:, :], Op.subtract)
        nc.scalar.activation(why[:, :], why[:, :], AF.Relu)
        inter = sbe.tile([128, NT], f32)
        nc.vector.tensor_tensor(inter[:, :], whx[:, :], why[:, :], Op.mult)
        den = sbe.tile([128, NT], f32)
        nc.vector.tensor_tensor(den[:, :], s_pre[:, :, 5], gta[:, :], Op.add)
        nc.vector.scalar_tensor_tensor(den[:, :], den[:, :], 1e-7, inter[:, :], Op.add, Op.subtract)
        nc.vector.reciprocal(den[:, :], den[:, :])
        miou = sbe.tile([128, NT], f32)
        nc.vector.tensor_tensor(miou[:, :], inter[:, :], den[:, :], Op.mult)
        nc.vector.tensor_tensor(miou[:, :], miou[:, :], w_all[:, :], Op.mult)

        # box loss: sum |s-t| over 4 coords * miou * w
        bsum = sbe.tile([128, NT], f32)
        bds = [sbe.tile([128, NT], f32, tag=f"bd{col}", name=f"bd{col}") for col in range(4)]
        for col in range(4):
            for c, (o, nn) in enumerate(GS_SPEC):
                T = slice(o, o + nn)
                nc.vector.tensor_tensor(bds[col][:, T], s_c[:, T, col], Gs[c][:, :, col], Op.subtract)
            nc.scalar.activation(bds[col][:, :], bds[col][:, :], AF.Abs)
        nc.vector.tensor_tensor(bsum[:, :], bds[0][:, :], bds[1][:, :], Op.add)
        nc.vector.tensor_tensor(bsum[:, :], bsum[:, :], bds[2][:, :], Op.add)
        nc.vector.tensor_tensor(bsum[:, :], bsum[:, :], bds[3][:, :], Op.add)
        nc.vector.tensor_tensor(bsum[:, :], bsum[:, :], miou[:, :], Op.mult)

        # conf loss: (s_conf - t_conf*miou)^2 * w   (miou already w-masked)
        cf = sbe.tile([128, NT], f32)
        for c, (o, nn) in enumerate(GS_SPEC):
            T = slice(o, o + nn)
            nc.vector.tensor_tensor(cf[:, T], Gs[c][:, :, 4], miou[:, T], Op.mult)
        nc.vector.tensor_tensor(cf[:, :], s_c[:, :, 4], cf[:, :], Op.subtract)
        nc.vector.tensor_tensor(cf[:, :], cf[:, :], cf[:, :], Op.mult)
        nc.vector.tensor_tensor(cf[:, :], cf[:, :], w_all[:, :], Op.mult)

        # reductions
        acc = sbe.tile([128, 4], f32)
        nc.vector.reduce_sum(acc[:, 0:1], kl[:, :], axis=AX.X)
        nc.vector.reduce_sum(acc[:, 1:2], bsum[:, :], axis=AX.X)
        nc.vector.reduce_sum(acc[:, 2:3], cf[:, :], axis=AX.X)
        nc.vector.reduce_sum(acc[:, 3:4], w_all[:, :], axis=AX.X)
        accp_full = ps.tile([1, 512], f32, tag="ps_acc", name="accp")
        accrow = accp_full[0:1, 0:4]
        nc.tensor.matmul(accrow[0:1, :], c_ones_col[:, 0:1], acc[:, :])
        res = sbe.tile([1, 8], f32)
        nc.vector.memset(res[:1, :], 0.0)
        nc.vector.tensor_copy(res[:1, 0:4], accrow[0:1, 0:4])
        Msafe = sbe.tile([1, 1], f32, tag="msafe")
        nc.vector.tensor_scalar(Msafe[:1, :], res[:1, 3:4], 1.0, None, Op.max)
        nc.vector.reciprocal(Msafe[:1, :], Msafe[:1, :])
        nc.vector.tensor_scalar(res[:1, 4:5], Msafe[:1, :], 1.0, None, Op.mult)
        nc.sync.dma_start(out.ap()[:, :], res[:1, :])

    nc.compile()
    return nc


def _consts():
    f32 = np.float32
    if "consts" not in _CACHE:
        _CACHE["consts"] = {
            "p1col": (np.arange(128, dtype=f32)[:, None] + 1.0),
            "ltmask": np.tril(np.ones((128, 128), f32), -1),
            "identity": np.eye(128, dtype=f32),
            "ones_col": np.ones((1, 128), f32),
            "ones128_col": np.ones((128, 1), f32),
            "kbig_lhs": np.full((128, 128), KILLV, np.float16),
        }
    return _CACHE["consts"]


def _prep_core_inputs(s_img, t_img):
    """Build per-core inputs from one (padded) student image [2048, 85] and
    the ORIGINAL teacher rows (1024 or 2048, uncompacted)."""
    f32 = np.float32
    W = _CACHE["W"]
    s = np.asarray(s_img, f32)
    t = np.asarray(t_img, f32)

    # teacher compaction (order-preserving), reference conf>0.5 + fallback
    mask = t[:, 4] > 0.5
    if not mask.any():
        mask = np.zeros_like(mask)
        mask[int(np.argmax(t[:, 4]))] = True
    vidx = np.where(mask)[0]
    nv = len(vidx)
    assert nv <= W, f"valid teachers {nv} > W={W}"
    tc = t[vidx]

    tx1 = tc[:, 0] - tc[:, 2] / f32(2); tx2 = tc[:, 0] + tc[:, 2] / f32(2)
    ty1 = tc[:, 1] - tc[:, 3] / f32(2); ty2 = tc[:, 1] + tc[:, 3] / f32(2)
    ta = ((tx2 - tx1) * (ty2 - ty1)).astype(f32)

    tcoords = np.zeros((5, W), np.float16)
    tcoords[0, :nv] = tx1; tcoords[1, :nv] = tx2
    tcoords[2, :nv] = ty1; tcoords[3, :nv] = ty2
    tcoords[4, :] = np.arange(W, dtype=f32)
    tcoords_rep = np.broadcast_to(tcoords[None, :, :], (128, 5, W)).copy()

    ta_row = np.full((1, W), 4.0, f32)
    ta_row[0, :nv] = ta
    inv_row = np.zeros((1, W), np.float16)
    inv_row[0, nv:] = 1.0

    t_rows_nat = np.zeros((W, D), f32)
    t_rows_nat[:nv] = tc

    s_cols = np.empty((128, NT, 5), f32)
    s_logits = np.empty((128, NT, 80), f32)
    for j in range(NT):
        s_cols[:, j, :] = s[j*128:(j+1)*128, :5]
        s_logits[:, j, :] = s[j*128:(j+1)*128, 5:]

    sx1 = s_cols[:, :, 0] - s_cols[:, :, 2] / 2
    sx2 = s_cols[:, :, 0] + s_cols[:, :, 2] / 2
    sy1 = s_cols[:, :, 1] - s_cols[:, :, 3] / 2
    sy2 = s_cols[:, :, 1] + s_cols[:, :, 3] / 2
    sa = ((sx2 - sx1) * (sy2 - sy1)).astype(f32)
    s_pre = np.stack([sx1, sx2, sy1, sy2, -sx1, sa, sa + f32(1e-7)], axis=-1).astype(f32)

    return {
        "tcoords": tcoords_rep, "ta_row": ta_row, "inv_row": inv_row,
        "s_cols": s_cols, "s_pre": s_pre, "s_logits": s_logits,
        "t_rows_nat": t_rows_nat, **_consts(),
    }, vidx


def _pad_scale1(s):
    """Pad students [1024, 85] -> [2048, 85] with inert rows."""
    f32 = np.float32
    ns = np.zeros((NS, D), f32)
    ns[:s.shape[0]] = s
    ns[s.shape[0]:, 0] = PAD_X
    ns[s.shape[0]:, 2] = 1.0
    ns[s.shape[0]:, 3] = 1.0
    return ns


def _max_nv(*teachers):
    best = 1
    for t in teachers:
        for b in range(t.shape[0]):
            best = max(best, int((t[b, :, 4] > 0.5).sum()))
    return best


def kernel(student_out0, teacher_out0, student_out1, teacher_out1):
    from concourse.bass_utils import run_bass_kernel_spmd

    student_out0 = np.asarray(student_out0, np.float32)
    teacher_out0 = np.asarray(teacher_out0, np.float32)
    student_out1 = np.asarray(student_out1, np.float32)
    teacher_out1 = np.asarray(teacher_out1, np.float32)

    W = (_max_nv(teacher_out0, teacher_out1) + 63) // 64 * 64
    if _CACHE.get("W") != W:
        _CACHE["W"] = W
        _CACHE["nc"] = _build_nc(W)
    nc = _CACHE["nc"]

    in_maps = []
    for c in range(4):
        m, _ = _prep_core_inputs(student_out0[c], teacher_out0[c])
        in_maps.append(m)
    for c in range(4):
        m, _ = _prep_core_inputs(_pad_scale1(student_out1[c]), teacher_out1[c])
        in_maps.append(m)

    res = run_bass_kernel_spmd(nc, in_maps, core_ids=list(range(8)))

    cls_t = box_t = conf_t = nm = np.float32(0.0)
    for c in range(8):
        o = res.results[c]["out"][0]
        kl_s, box_s, conf_s, M, minv = o[0], o[1], o[2], o[3], o[4]
        cls_t += np.float32(kl_s) * np.float32(minv) * np.float32(TEMP * TEMP)
        box_t += np.float32(box_s) * np.float32(minv) / np.float32(4.0)
        conf_t += np.float32(conf_s) * np.float32(minv)
        nm += np.float32(M)
    nms = max(nm, np.float32(1.0))
    cls_t, box_t, conf_t = cls_t / nms, box_t / nms, conf_t / nms
    total = np.float32(ALPHA) * cls_t + np.float32(BETA) * box_t + np.float32(1.0 - ALPHA - BETA) * conf_t
    return np.float32(total)
